# revision 21
# baseline (speedup 1.0000x reference)
"""Capsule-network kernel for 8x TRN2 NeuronCores (data-parallel over batch).

Reference computation (see problem):
  prim = primary_input.reshape(B, 8, 1024)
  prev = zeros(B, 4096)
  for col in 0..3:
    # layer0: inp = [prim_t, x_t, col] (1537) @ W0 -> relu -> flat -> roll(-128)
    # layer1: inp = [x_t, col] (513) @ W1 -> relu -> flat -> roll(+128)
  out = prev @ W_out + b_out

Kernel strategy (per core, batch shard Bc=512):
  - Everything on-chip is FEATURE-MAJOR: tiles are [128 features, Bc batch].
    ROLL=128 == partition count, so rolls are free tile re-indexings.
  - The scalar `col` concat input contributes col*W[last_row] to the
    pre-activation -> folded into per-col biases (computed on host).
  - P = prim @ W0[0:1024] is col-invariant -> computed once (phase 1),
    kept in SBUF as bf16, added during the layer0 drain each col.
  - col 0 layer0 has x=0 -> out = relu(P + b0): no matmuls at all.
  - Matmuls run as bf16 (same 1 col/cycle PE rate as fp32r, but
    the 2-byte LDWEIGHTS hides fully: measured cadence ~216ns vs
    ~227ns for fp32r).  Activations/weights bf16, psum fp32.
  - HW measurement: an fp32r matmul whose stationary weights differ from
    the previous matmul costs ~252ns; same-weights runs cost ~226.7ns.
    So layers are swept (fo, k) outer / row t inner: 8 consecutive
    matmuls share one weight tile (one sweep = 8 psum banks).
  - 6 dummy matmuls at t=0 (on a memset tile) ramp the PE out of its
    low p-state during the initial DMA wait.
"""

import numpy as np

# ---- problem constants (hardcoded; kernel.py must be self-contained) ----
B_FULL = 4096
D_IN = 8192
T = 8            # NUM_TALL
NW = 4           # NUM_WIDE
F = 512          # feature size per capsule row
ROLL = 128
N_CORES = 8
BC = B_FULL // N_CORES   # per-core batch = 512
S = (F * T) // 128       # state feature tiles = 32
KP = (D_IN // T) // 128  # prim k-tiles per capsule row = 8
KX = F // 128            # x k-tiles = 4
FO = F // 128            # output feature tiles per row-layer = 4
N_OUT = 10
N_WARM = 6               # dummy p-state warmup matmuls

_CACHE = {}


def _build_program():
    """Build (and cache) the single-core Bass program. Same program runs
    SPMD on all 8 cores with different batch shards."""
    if "nc" in _CACHE:
        return _CACHE["nc"], _CACHE["names"]

    from contextlib import ExitStack

    import concourse.tile as tile
    from concourse import bacc, mybir

    f32 = mybir.dt.float32
    f32r = mybir.dt.float32r
    bf16 = mybir.dt.bfloat16
    AF = mybir.ActivationFunctionType
    ADD = mybir.AluOpType.add

    nc = bacc.Bacc("TRN2", target_bir_lowering=False, debug=False,
                   num_devices=N_CORES)

    # prim + w0p travel and multiply as bf16: phase-1 is the only
    # DMA-heavy span (16.8MB of prim in fp32 saturates the 360 GB/s DMA
    # system during the cold start); halving the bytes costs ~26ns/matmul
    # of bf16 LDWEIGHTS overhead on the 256 P-matmuls but removes all
    # DMA-starvation stalls.
    prim_d = nc.dram_tensor("prim_t", [D_IN, BC], bf16, kind="ExternalInput").ap()
    w0p_d = nc.dram_tensor("w0p", [KP * 128, F], bf16, kind="ExternalInput").ap()
    w0x_d = nc.dram_tensor("w0x", [F, F], bf16, kind="ExternalInput").ap()
    w1x_d = nc.dram_tensor("w1x", [F, F], bf16, kind="ExternalInput").ap()
    wout_d = nc.dram_tensor("wout_packed", [128, S * N_OUT], bf16,
                            kind="ExternalInput").ap()
    bias0_d = nc.dram_tensor("bias0", [128, NW * FO], f32, kind="ExternalInput").ap()
    bias1_d = nc.dram_tensor("bias1", [128, NW * FO], f32, kind="ExternalInput").ap()
    bout_d = nc.dram_tensor("bout", [N_OUT, 1], f32, kind="ExternalInput").ap()
    out_d = nc.dram_tensor("out", [N_OUT, BC], f32, kind="ExternalOutput").ap()

    with tile.TileContext(nc) as tc, ExitStack() as ctx:
        const = ctx.enter_context(tc.tile_pool(name="const", bufs=1))
        state = ctx.enter_context(tc.tile_pool(name="state", bufs=1))
        cpool = ctx.enter_context(tc.tile_pool(name="cpool", bufs=33))
        prim_pool = ctx.enter_context(tc.tile_pool(name="primp", bufs=12))
        ppool = ctx.enter_context(tc.tile_pool(name="psum", bufs=8, space="PSUM"))

        # ---- constants ----
        w0p_sb = [const.tile([128, F], bf16, name=f"w0p{k}", tag=f"w0p{k}")
                  for k in range(KP)]
        w0x_sb = [const.tile([128, F], bf16, name=f"w0x{k}", tag=f"w0x{k}")
                  for k in range(KX)]
        w1x_sb = [const.tile([128, F], bf16, name=f"w1x{k}", tag=f"w1x{k}")
                  for k in range(KX)]
        wout_sb = const.tile([128, S * N_OUT], bf16, name="wout_sb", tag="wout")
        bias0_sb = const.tile([128, NW * FO], f32, name="bias0_sb", tag="bias0")
        bias1_sb = const.tile([128, NW * FO], f32, name="bias1_sb", tag="bias1")
        bout_sb = const.tile([N_OUT, 1], f32, name="bout_sb", tag="bout")
        warm_sb = const.tile([128, 128], f32, name="warm_sb", tag="warm")

        # ---- persistent state ----
        A = [state.tile([128, BC], bf16, name=f"state_a{i}", tag=f"A{i}")
             for i in range(S)]
        P = [state.tile([128, BC], bf16, name=f"state_p{i}", tag=f"P{i}")
             for i in range(S)]

        # ---- p-state warmup: dummy matmuls on a memset tile ----
        # (fp32 runs at 4 cycles/row so a 128-wide moving dim gives
        # ~213-790ns per dummy across the ramp)
        nc.vector.memset(warm_sb[:], 0.0)
        for i in range(N_WARM):
            ps = ppool.tile([128, BC], f32, name=f"warm{i}", tag="mm")
            nc.tensor.matmul(ps[0:128, 0:128], warm_sb[:], warm_sb[:],
                             start=True, stop=True)

        def load_deferred_consts(gi):
            # late-needed constants ride the idle gpsimd (SWDGE) queue so
            # the sync/scalar queues keep streaming prim.  w1x must be
            # resident by ~15us (first col0-layer1 chunk), so it goes out
            # at gi=0 -- the gpsimd queue has nothing else and issues it
            # at t~1us.
            if gi == 0:
                nc.sync.dma_start(bias0_sb[:], bias0_d[:, :])
                nc.gpsimd.dma_start(bias1_sb[:], bias1_d[:, :])
                nc.gpsimd.dma_start(bout_sb[:], bout_d[:, :])
                for k in range(KX):
                    nc.gpsimd.dma_start(w1x_sb[k][:], w1x_d[k * 128:(k + 1) * 128, :])
            elif gi == 1:
                for k in range(KX):
                    nc.gpsimd.dma_start(w0x_sb[k][:], w0x_d[k * 128:(k + 1) * 128, :])
            elif gi == 5:
                nc.gpsimd.dma_start(wout_sb[:], wout_d[:, :])

        C = [None] * S  # col-current layer0 outputs (cpool ring tiles)

        # ==== phase 1: P[j] = prim @ W0p (col-invariant), fused col0-L0 ====
        # Per-row groups (4 psum banks each): bf16 LDWEIGHTS hides fully
        # so weight-run batching is unnecessary, and small groups smooth
        # the psum-drain handoff between groups.
        # Drains: P copy (DVE, bf16) + col0-L0 C = relu(P+b0) (ACT).
        groups = [(t,) for t in range(T)]

        def prim_dma(t, k):
            # one k-tile per DMA, full-width: the DMA system is
            # descriptor-line-rate limited, so 1KB lines (full bf16 rows)
            # move twice the bytes per line vs split halves.  Row 0 and
            # odd rows ride the scalar queue (it starts issuing ~5us
            # before sync, which carries the TileContext preamble).
            g = t * KP + k
            tile_ = prim_pool.tile([128, BC], bf16, name=f"prim_{g}",
                                   tag="prim")
            # scalar (ACT engine: also runs drains) only carries row 0,
            # interleaved with w0p.  Sync takes rows 1/2/4/6 (row 1 first,
            # right after bias0, so it lands before its ~14us deadline);
            # gpsimd takes rows 3/5/7 after the small deferred consts.
            if t == 0:
                q = nc.scalar
            elif t in (1, 2, 4, 6):
                q = nc.sync
            else:
                q = nc.gpsimd
            q.dma_start(tile_[:], prim_d[g * 128:(g + 1) * 128, :])
            return tile_

        def layer1_chunk(c, rows):
            # layer1 for a subset of rows (weight run-of-len(rows)).
            # A[4t+fo] = relu(W1x.T C + b1c);  C k-tile = C[(4t+k+1)%S]
            for fo in range(FO):
                pss = {t: ppool.tile([128, BC], f32, name=f"ps1_{c}_{fo}_{t}",
                                     tag="mm") for t in rows}
                for k in range(KX):
                    w_ap = w1x_sb[k][:, fo * 128:(fo + 1) * 128]
                    for t in rows:
                        nc.tensor.matmul(
                            pss[t][:], w_ap, C[(t * FO + k + 1) % S][:],
                            start=(k == 0), stop=(k == KX - 1))
                b1ap = bias1_sb[:, c * FO + fo:c * FO + fo + 1]
                for t in rows:
                    j = t * FO + fo
                    if t % 2 == 0:
                        nc.scalar.activation(A[j][:], pss[t][:], AF.Relu,
                                             bias=b1ap)
                    else:
                        # relu(psum + bias) on DVE: (psum add bias) max 0
                        nc.vector.tensor_scalar(A[j][:], pss[t][:], b1ap, 0.0,
                                                ADD, mybir.AluOpType.max)

        for gi, grp in enumerate(groups):
            pss = {}
            for t in grp:
                for fo in range(FO):
                    pss[(t, fo)] = ppool.tile([128, BC], f32,
                                              name=f"ps_p1_{t}_{fo}", tag="mm")
            pt = {}
            if gi == 0:
                # interleave w0p with row 0's tiles on scalar so the
                # k-th matmul's pair (w0p[k], prim(0,k)) lands together
                for k in range(KP):
                    nc.scalar.dma_start(w0p_sb[k][:],
                                        w0p_d[k * 128:(k + 1) * 128, :])
                    pt[(0, k)] = prim_dma(0, k)
            else:
                for k in range(KP):
                    for t in grp:
                        pt[(t, k)] = prim_dma(t, k)
            for k in range(KP):
                for fo in range(FO):
                    for t in grp:
                        nc.tensor.matmul(
                            pss[(t, fo)][:],
                            w0p_sb[k][:, fo * 128:(fo + 1) * 128],
                            pt[(t, k)][:],
                            start=(k == 0), stop=(k == KP - 1))
            load_deferred_consts(gi)
            for t in grp:
                for fo in range(FO):
                    j = t * FO + fo
                    nc.vector.tensor_copy(P[j][:], pss[(t, fo)][:])
                    ct = cpool.tile([128, BC], bf16, name=f"c0_{j}", tag="C")
                    nc.scalar.activation(ct[:], pss[(t, fo)][:], AF.Relu,
                                         bias=bias0_sb[:, fo:fo + 1])
                    C[j] = ct
        # col-0 layer1 with full run-of-8 weight reuse (bf16 phase-1 DMA
        # leaves enough bandwidth slack that no absorber work is needed)
        layer1_chunk(0, tuple(range(T)))

        # ==== layer emitters: (fo, k) outer, t inner -> weight run-of-8 ====
        def layer0_col(c):
            # C[4t+fo] = relu(W0x.T x + P + b0c);  x k-tile = A[(4t+k-1)%S]
            # The t sweep starts at t=1: the k=0 input A[4t-1] is a fo3
            # tile of the previous col's layer1, and t=0 needs A[31] --
            # the very LAST drain of that col.  Rotating gives each A
            # one extra sweep-step of drain slack.
            rows = [(1 + i) % T for i in range(T)]
            for fo in range(FO):
                pss = {t: ppool.tile([128, BC], f32, name=f"ps0_{c}_{fo}_{t}",
                                     tag="mm") for t in rows}
                for k in range(KX):
                    w_ap = w0x_sb[k][:, fo * 128:(fo + 1) * 128]
                    for t in rows:
                        nc.tensor.matmul(
                            pss[t][:], w_ap, A[(t * FO + k - 1) % S][:],
                            start=(k == 0), stop=(k == KX - 1))
                b0ap = bias0_sb[:, c * FO + fo:c * FO + fo + 1]
                for t in rows:
                    j = t * FO + fo
                    ct = cpool.tile([128, BC], bf16, name=f"c{c}_{j}", tag="C")
                    # ct = (psum + bias0_c) + P  on DVE, then relu on ACT
                    nc.vector.scalar_tensor_tensor(
                        ct[:], pss[t][:], b0ap, P[j][:], ADD, ADD)
                    nc.scalar.activation(ct[:], ct[:], AF.Relu)
                    C[j] = ct

        def layer1_col(c):
            # A[4t+fo] = relu(W1x.T C + b1c);  C k-tile = C[(4t+k+1)%S]
            for fo in range(FO):
                pss = [ppool.tile([128, BC], f32, name=f"ps1_{c}_{fo}_{t}",
                                  tag="mm") for t in range(T)]
                for k in range(KX):
                    w_ap = w1x_sb[k][:, fo * 128:(fo + 1) * 128]
                    for t in range(T):
                        nc.tensor.matmul(
                            pss[t][:], w_ap, C[(t * FO + k + 1) % S][:],
                            start=(k == 0), stop=(k == KX - 1))
                b1ap = bias1_sb[:, c * FO + fo:c * FO + fo + 1]
                for t in range(T):
                    j = t * FO + fo
                    if t % 2 == 0:
                        nc.scalar.activation(A[j][:], pss[t][:], AF.Relu,
                                             bias=b1ap)
                    else:
                        # relu(psum + bias) on DVE: (psum add bias) max 0
                        nc.vector.tensor_scalar(A[j][:], pss[t][:], b1ap, 0.0,
                                                ADD, mybir.AluOpType.max)

        # ==== cols 1..3 (col-0 layer1 was interleaved into phase 1) ====
        for c in range(1, NW):
            layer0_col(c)
            layer1_col(c)

        # ---- final: out = prev @ W_out + b_out;  prev[k] = A[(k-1) % S] ----
        psf_full = ppool.tile([128, BC], f32, name="psf", tag="mm")
        psf = psf_full[0:N_OUT, :]
        # emit in col-3's A-write order (sweep fo, then t) so the
        # accumulation chain chases the layer1 drains
        n = 0
        for fo in range(FO):
            for t in range(T):
                k = (t * FO + fo + 1) % S
                nc.tensor.matmul(
                    psf[:],
                    wout_sb[:, k * N_OUT:(k + 1) * N_OUT],
                    A[(k - 1) % S][:],
                    start=(n == 0), stop=(n == S - 1))
                n += 1
        out_sb = cpool.tile([N_OUT, BC], f32, name="out_sb", tag="C")
        nc.scalar.activation(out_sb[:], psf[:], AF.Identity, bias=bout_sb[:])
        nc.sync.dma_start(out_d[:, :], out_sb[:])

    nc.compile()

    names = dict(prim="prim_t", w0p="w0p", w0x="w0x", w1x="w1x",
                 wout="wout_packed", bias0="bias0", bias1="bias1",
                 bout="bout", out="out")
    _CACHE["nc"] = nc
    _CACHE["names"] = names
    return nc, names


def _make_in_maps(primary_input, W0, b0, W1, b1, W_out, b_out):
    """Host-side sharding + layout prep (all cheap numpy except the
    feature-major transpose of the batch shards)."""
    primary_input = np.ascontiguousarray(primary_input, dtype=np.float32)
    W0 = np.asarray(W0, dtype=np.float32)
    b0 = np.asarray(b0, dtype=np.float32)
    W1 = np.asarray(W1, dtype=np.float32)
    b1 = np.asarray(b1, dtype=np.float32)
    W_out = np.asarray(W_out, dtype=np.float32)
    b_out = np.asarray(b_out, dtype=np.float32)

    import ml_dtypes
    ps = D_IN // T  # 1024
    w0p = np.ascontiguousarray(W0[:ps].astype(ml_dtypes.bfloat16))  # [1024, 512]
    w0x = np.ascontiguousarray(W0[ps:ps + F].astype(ml_dtypes.bfloat16))
    w0_last = W0[ps + F]                             # [512]
    w1x = np.ascontiguousarray(W1[:F].astype(ml_dtypes.bfloat16))
    w1_last = W1[F]                                  # [512]

    bias0 = np.concatenate(
        [(b0 + c * w0_last).reshape(FO, 128).T for c in range(NW)], axis=1)
    bias1 = np.concatenate(
        [(b1 + c * w1_last).reshape(FO, 128).T for c in range(NW)], axis=1)
    bias0 = np.ascontiguousarray(bias0, dtype=np.float32)   # [128, 16]
    bias1 = np.ascontiguousarray(bias1, dtype=np.float32)   # [128, 16]

    # wout_packed[p, k*10+o] = W_out[128k+p, o]
    wout_packed = np.ascontiguousarray(
        W_out.reshape(S, 128, N_OUT).transpose(1, 0, 2).reshape(128, S * N_OUT)
        .astype(ml_dtypes.bfloat16))
    bout = np.ascontiguousarray(b_out.reshape(N_OUT, 1))

    shared = dict(w0p=w0p, w0x=w0x, w1x=w1x, wout_packed=wout_packed,
                  bias0=bias0, bias1=bias1, bout=bout)
    in_maps = []
    for core in range(N_CORES):
        shard = primary_input[core * BC:(core + 1) * BC]          # [512, 8192]
        prim_t = np.ascontiguousarray(shard.T.astype(ml_dtypes.bfloat16))
        m = {"prim_t": prim_t}
        m.update(shared)
        in_maps.append(m)
    return in_maps


def _install_ntff_hook():
    """Provide antenv.axon_hooks (absent in this image) backed by ctypes
    calls into libaxon_pjrt.so, so run_bass_kernel_spmd(trace=True) can
    capture NTFF profiles. Mirrors trn_agent_boot.trn_boot."""
    import contextlib
    import ctypes
    import sys
    import types

    if "antenv.axon_hooks" in sys.modules:
        return
    so_path = "/opt/axon/libaxon_pjrt.so"
    lib = ctypes.CDLL(so_path)
    lib.axon_start_nrt_profile.argtypes = [ctypes.POINTER(ctypes.c_int64),
                                           ctypes.c_size_t]
    lib.axon_start_nrt_profile.restype = ctypes.c_int64
    lib.axon_stop_nrt_profile.argtypes = [ctypes.c_char_p]
    lib.axon_stop_nrt_profile.restype = ctypes.c_int64

    @contextlib.contextmanager
    def _hook(output_dir, device_ids):
        import jax
        jax.devices()
        if device_ids:
            ids = (ctypes.c_int64 * len(device_ids))(*device_ids)
            rc = lib.axon_start_nrt_profile(ids, len(device_ids))
        else:
            rc = lib.axon_start_nrt_profile(None, 0)
        if rc != 0:
            raise RuntimeError(f"axon_start_nrt_profile rc={rc}")
        try:
            yield
        finally:
            n = lib.axon_stop_nrt_profile(str(output_dir).encode())
            print(f"profile: {n} file(s) written to {output_dir}",
                  file=sys.stderr)

    mod = types.ModuleType("antenv.axon_hooks")
    mod.get_axon_ntff_profile_hook = lambda: _hook
    mod.set_axon_ntff_profile_hook = lambda h: None
    sys.modules["antenv.axon_hooks"] = mod
    import antenv
    antenv.axon_hooks = mod


def kernel(primary_input, W0, b0, W1, b1, W_out, b_out, _trace=False,
           _trace_cores=None):
    from concourse import bass_utils

    if _trace:
        _install_ntff_hook()

    nc, _ = _build_program()
    in_maps = _make_in_maps(primary_input, W0, b0, W1, b1, W_out, b_out)
    res = bass_utils.run_bass_kernel_spmd(
        nc, in_maps, core_ids=list(range(N_CORES)),
        trace=_trace, trace_cores=_trace_cores)
    out = np.empty((B_FULL, N_OUT), dtype=np.float32)
    for core in range(N_CORES):
        out[core * BC:(core + 1) * BC] = res.results[core]["out"].T
    if _trace:
        kernel._last_results = res
    return out



# revision 23
# speedup vs baseline: 1.0032x; 1.0032x over previous
"""Capsule-network kernel for 8x TRN2 NeuronCores (data-parallel over batch).

Reference computation (see problem):
  prim = primary_input.reshape(B, 8, 1024)
  prev = zeros(B, 4096)
  for col in 0..3:
    # layer0: inp = [prim_t, x_t, col] (1537) @ W0 -> relu -> flat -> roll(-128)
    # layer1: inp = [x_t, col] (513) @ W1 -> relu -> flat -> roll(+128)
  out = prev @ W_out + b_out

Kernel strategy (per core, batch shard Bc=512):
  - Everything on-chip is FEATURE-MAJOR: tiles are [128 features, Bc batch].
    ROLL=128 == partition count, so rolls are free tile re-indexings.
  - The scalar `col` concat input contributes col*W[last_row] to the
    pre-activation -> folded into per-col biases (computed on host).
  - P = prim @ W0[0:1024] is col-invariant -> computed once (phase 1),
    kept in SBUF as bf16, added during the layer0 drain each col.
  - col 0 layer0 has x=0 -> out = relu(P + b0): no matmuls at all.
  - Matmuls run as bf16 (same 1 col/cycle PE rate as fp32r, but
    the 2-byte LDWEIGHTS hides fully: measured cadence ~216ns vs
    ~227ns for fp32r).  Activations/weights bf16, psum fp32.
  - HW measurement: an fp32r matmul whose stationary weights differ from
    the previous matmul costs ~252ns; same-weights runs cost ~226.7ns.
    So layers are swept (fo, k) outer / row t inner: 8 consecutive
    matmuls share one weight tile (one sweep = 8 psum banks).
  - 6 dummy matmuls at t=0 (on a memset tile) ramp the PE out of its
    low p-state during the initial DMA wait.
"""

import numpy as np

# ---- problem constants (hardcoded; kernel.py must be self-contained) ----
B_FULL = 4096
D_IN = 8192
T = 8            # NUM_TALL
NW = 4           # NUM_WIDE
F = 512          # feature size per capsule row
ROLL = 128
N_CORES = 8
BC = B_FULL // N_CORES   # per-core batch = 512
S = (F * T) // 128       # state feature tiles = 32
KP = (D_IN // T) // 128  # prim k-tiles per capsule row = 8
KX = F // 128            # x k-tiles = 4
FO = F // 128            # output feature tiles per row-layer = 4
N_OUT = 10
N_WARM = 6               # dummy p-state warmup matmuls

_CACHE = {}


def _build_program():
    """Build (and cache) the single-core Bass program. Same program runs
    SPMD on all 8 cores with different batch shards."""
    if "nc" in _CACHE:
        return _CACHE["nc"], _CACHE["names"]

    from contextlib import ExitStack

    import concourse.tile as tile
    from concourse import bacc, mybir

    f32 = mybir.dt.float32
    f32r = mybir.dt.float32r
    bf16 = mybir.dt.bfloat16
    AF = mybir.ActivationFunctionType
    ADD = mybir.AluOpType.add

    nc = bacc.Bacc("TRN2", target_bir_lowering=False, debug=False,
                   num_devices=N_CORES)

    # prim + w0p travel and multiply as bf16: phase-1 is the only
    # DMA-heavy span (16.8MB of prim in fp32 saturates the 360 GB/s DMA
    # system during the cold start); halving the bytes costs ~26ns/matmul
    # of bf16 LDWEIGHTS overhead on the 256 P-matmuls but removes all
    # DMA-starvation stalls.
    prim_d = nc.dram_tensor("prim_t", [D_IN, BC], bf16, kind="ExternalInput").ap()
    w0p_d = nc.dram_tensor("w0p", [KP * 128, F], bf16, kind="ExternalInput").ap()
    w0x_d = nc.dram_tensor("w0x", [F, F], bf16, kind="ExternalInput").ap()
    w1x_d = nc.dram_tensor("w1x", [F, F], bf16, kind="ExternalInput").ap()
    wout_d = nc.dram_tensor("wout_packed", [128, S * N_OUT], bf16,
                            kind="ExternalInput").ap()
    bias0_d = nc.dram_tensor("bias0", [128, NW * FO], f32, kind="ExternalInput").ap()
    bias1_d = nc.dram_tensor("bias1", [128, NW * FO], f32, kind="ExternalInput").ap()
    bout_d = nc.dram_tensor("bout", [N_OUT, 1], f32, kind="ExternalInput").ap()
    out_d = nc.dram_tensor("out", [N_OUT, BC], f32, kind="ExternalOutput").ap()

    with tile.TileContext(nc) as tc, ExitStack() as ctx:
        const = ctx.enter_context(tc.tile_pool(name="const", bufs=1))
        state = ctx.enter_context(tc.tile_pool(name="state", bufs=1))
        cpool = ctx.enter_context(tc.tile_pool(name="cpool", bufs=33))
        prim_pool = ctx.enter_context(tc.tile_pool(name="primp", bufs=12))
        ppool = ctx.enter_context(tc.tile_pool(name="psum", bufs=8, space="PSUM"))

        # ---- constants ----
        w0p_sb = [const.tile([128, F], bf16, name=f"w0p{k}", tag=f"w0p{k}")
                  for k in range(KP)]
        w0x_sb = [const.tile([128, F], bf16, name=f"w0x{k}", tag=f"w0x{k}")
                  for k in range(KX)]
        w1x_sb = [const.tile([128, F], bf16, name=f"w1x{k}", tag=f"w1x{k}")
                  for k in range(KX)]
        wout_sb = const.tile([128, S * N_OUT], bf16, name="wout_sb", tag="wout")
        bias0_sb = const.tile([128, NW * FO], f32, name="bias0_sb", tag="bias0")
        bias1_sb = const.tile([128, NW * FO], f32, name="bias1_sb", tag="bias1")
        bout_sb = const.tile([N_OUT, 1], f32, name="bout_sb", tag="bout")
        warm_sb = const.tile([128, 128], f32, name="warm_sb", tag="warm")

        # ---- persistent state ----
        A = [state.tile([128, BC], bf16, name=f"state_a{i}", tag=f"A{i}")
             for i in range(S)]
        P = [state.tile([128, BC], bf16, name=f"state_p{i}", tag=f"P{i}")
             for i in range(S)]

        # ---- p-state warmup: dummy matmuls on a memset tile ----
        # (fp32 runs at 4 cycles/row so a 128-wide moving dim gives
        # ~213-790ns per dummy across the ramp)
        nc.vector.memset(warm_sb[:], 0.0)
        for i in range(N_WARM):
            ps = ppool.tile([128, BC], f32, name=f"warm{i}", tag="mm")
            nc.tensor.matmul(ps[0:128, 0:128], warm_sb[:], warm_sb[:],
                             start=True, stop=True)

        def load_deferred_consts(gi):
            # late-needed constants ride the idle gpsimd (SWDGE) queue so
            # the sync/scalar queues keep streaming prim.  w1x must be
            # resident by ~15us (first col0-layer1 chunk), so it goes out
            # at gi=0 -- the gpsimd queue has nothing else and issues it
            # at t~1us.
            if gi == 0:
                nc.sync.dma_start(bias0_sb[:], bias0_d[:, :])
                nc.gpsimd.dma_start(bias1_sb[:], bias1_d[:, :])
                nc.gpsimd.dma_start(bout_sb[:], bout_d[:, :])
                for k in range(KX):
                    nc.gpsimd.dma_start(w1x_sb[k][:], w1x_d[k * 128:(k + 1) * 128, :])
            elif gi == 1:
                for k in range(KX):
                    nc.gpsimd.dma_start(w0x_sb[k][:], w0x_d[k * 128:(k + 1) * 128, :])
            elif gi == 5:
                nc.gpsimd.dma_start(wout_sb[:], wout_d[:, :])

        C = [None] * S  # col-current layer0 outputs (cpool ring tiles)

        # ==== phase 1: P[j] = prim @ W0p (col-invariant), fused col0-L0 ====
        # Per-row groups (4 psum banks each): bf16 LDWEIGHTS hides fully
        # so weight-run batching is unnecessary, and small groups smooth
        # the psum-drain handoff between groups.
        # Drains: P copy (DVE, bf16) + col0-L0 C = relu(P+b0) (ACT).
        groups = [(t,) for t in range(T)]

        def prim_dma(t, k):
            # one k-tile per DMA, full-width: the DMA system is
            # descriptor-line-rate limited, so 1KB lines (full bf16 rows)
            # move twice the bytes per line vs split halves.  Row 0 and
            # odd rows ride the scalar queue (it starts issuing ~5us
            # before sync, which carries the TileContext preamble).
            g = t * KP + k
            tile_ = prim_pool.tile([128, BC], bf16, name=f"prim_{g}",
                                   tag="prim")
            # scalar (ACT engine: also runs drains) only carries row 0,
            # interleaved with w0p.  Sync takes rows 1/2/4/6 (row 1 first,
            # right after bias0, so it lands before its ~14us deadline);
            # gpsimd takes rows 3/5/7 after the small deferred consts.
            if t == 0:
                q = nc.scalar
            elif t in (1, 2, 4, 6):
                q = nc.sync
            else:
                q = nc.gpsimd
            q.dma_start(tile_[:], prim_d[g * 128:(g + 1) * 128, :])
            return tile_

        def layer1_chunk(c, rows):
            # layer1 for a subset of rows (weight run-of-len(rows)).
            # A[4t+fo] = relu(W1x.T C + b1c);  C k-tile = C[(4t+k+1)%S]
            for fo in range(FO):
                pss = {t: ppool.tile([128, BC], f32, name=f"ps1_{c}_{fo}_{t}",
                                     tag="mm") for t in rows}
                for k in range(KX):
                    w_ap = w1x_sb[k][:, fo * 128:(fo + 1) * 128]
                    for t in rows:
                        nc.tensor.matmul(
                            pss[t][:], w_ap, C[(t * FO + k + 1) % S][:],
                            start=(k == 0), stop=(k == KX - 1))
                b1ap = bias1_sb[:, c * FO + fo:c * FO + fo + 1]
                for t in rows:
                    j = t * FO + fo
                    if t % 2 == 0:
                        nc.scalar.activation(A[j][:], pss[t][:], AF.Relu,
                                             bias=b1ap)
                    else:
                        # relu(psum + bias) on DVE: (psum add bias) max 0
                        nc.vector.tensor_scalar(A[j][:], pss[t][:], b1ap, 0.0,
                                                ADD, mybir.AluOpType.max)

        for gi, grp in enumerate(groups):
            pss = {}
            for t in grp:
                for fo in range(FO):
                    pss[(t, fo)] = ppool.tile([128, BC], f32,
                                              name=f"ps_p1_{t}_{fo}", tag="mm")
            pt = {}
            if gi == 0:
                # interleave w0p with row 0's tiles on scalar so the
                # k-th matmul's pair (w0p[k], prim(0,k)) lands together
                for k in range(KP):
                    nc.scalar.dma_start(w0p_sb[k][:],
                                        w0p_d[k * 128:(k + 1) * 128, :])
                    pt[(0, k)] = prim_dma(0, k)
            else:
                for k in range(KP):
                    for t in grp:
                        pt[(t, k)] = prim_dma(t, k)
            for k in range(KP):
                for fo in range(FO):
                    for t in grp:
                        nc.tensor.matmul(
                            pss[(t, fo)][:],
                            w0p_sb[k][:, fo * 128:(fo + 1) * 128],
                            pt[(t, k)][:],
                            start=(k == 0), stop=(k == KP - 1))
            load_deferred_consts(gi)
            for t in grp:
                for fo in range(FO):
                    j = t * FO + fo
                    nc.vector.tensor_copy(P[j][:], pss[(t, fo)][:])
                    ct = cpool.tile([128, BC], bf16, name=f"c0_{j}", tag="C")
                    nc.scalar.activation(ct[:], pss[(t, fo)][:], AF.Relu,
                                         bias=bias0_sb[:, fo:fo + 1])
                    C[j] = ct
        # col-0 layer1 with full run-of-8 weight reuse (bf16 phase-1 DMA
        # leaves enough bandwidth slack that no absorber work is needed)
        layer1_chunk(0, tuple(range(T)))

        # ==== layer emitters: (fo, k) outer, t inner -> weight run-of-8 ====
        def layer0_col(c):
            # C[4t+fo] = relu(W0x.T x + P + b0c);  x k-tile = A[(4t+k-1)%S]
            # The t sweep starts at t=1: the k=0 input A[4t-1] is a fo3
            # tile of the previous col's layer1, and t=0 needs A[31] --
            # the very LAST drain of that col.  Rotating gives each A
            # one extra sweep-step of drain slack.
            rows = [(1 + i) % T for i in range(T)]
            for fo in range(FO):
                pss = {t: ppool.tile([128, BC], f32, name=f"ps0_{c}_{fo}_{t}",
                                     tag="mm") for t in rows}
                for k in range(KX):
                    w_ap = w0x_sb[k][:, fo * 128:(fo + 1) * 128]
                    for t in rows:
                        nc.tensor.matmul(
                            pss[t][:], w_ap, A[(t * FO + k - 1) % S][:],
                            start=(k == 0), stop=(k == KX - 1))
                b0ap = bias0_sb[:, c * FO + fo:c * FO + fo + 1]
                for t in rows:
                    j = t * FO + fo
                    ct = cpool.tile([128, BC], bf16, name=f"c{c}_{j}", tag="C")
                    # ct = (psum + bias0_c) + P  on DVE, then relu on ACT
                    nc.vector.scalar_tensor_tensor(
                        ct[:], pss[t][:], b0ap, P[j][:], ADD, ADD)
                    nc.scalar.activation(ct[:], ct[:], AF.Relu)
                    C[j] = ct

        def layer1_col(c):
            # A[4t+fo] = relu(W1x.T C + b1c);  C k-tile = C[(4t+k+1)%S]
            for fo in range(FO):
                pss = [ppool.tile([128, BC], f32, name=f"ps1_{c}_{fo}_{t}",
                                  tag="mm") for t in range(T)]
                for k in range(KX):
                    w_ap = w1x_sb[k][:, fo * 128:(fo + 1) * 128]
                    for t in range(T):
                        nc.tensor.matmul(
                            pss[t][:], w_ap, C[(t * FO + k + 1) % S][:],
                            start=(k == 0), stop=(k == KX - 1))
                b1ap = bias1_sb[:, c * FO + fo:c * FO + fo + 1]
                for t in range(T):
                    j = t * FO + fo
                    if t % 2 == 0:
                        nc.scalar.activation(A[j][:], pss[t][:], AF.Relu,
                                             bias=b1ap)
                    else:
                        # relu(psum + bias) on DVE: (psum add bias) max 0
                        nc.vector.tensor_scalar(A[j][:], pss[t][:], b1ap, 0.0,
                                                ADD, mybir.AluOpType.max)

        # ==== cols 1..3 (col-0 layer1 was interleaved into phase 1) ====
        for c in range(1, NW):
            layer0_col(c)
            layer1_col(c)

        # ---- final: out = prev @ W_out + b_out;  prev[k] = A[(k-1) % S] ----
        psf_full = ppool.tile([128, BC], f32, name="psf", tag="mm")
        psf = psf_full[0:N_OUT, :]
        # emit in col-3's A-write order (sweep fo, then t) so the
        # accumulation chain chases the layer1 drains
        n = 0
        for fo in range(FO):
            for t in range(T):
                k = (t * FO + fo + 1) % S
                nc.tensor.matmul(
                    psf[:],
                    wout_sb[:, k * N_OUT:(k + 1) * N_OUT],
                    A[(k - 1) % S][:],
                    start=(n == 0), stop=(n == S - 1))
                n += 1
        out_sb = cpool.tile([N_OUT, BC], f32, name="out_sb", tag="C")
        nc.scalar.activation(out_sb[:], psf[:], AF.Identity, bias=bout_sb[:])
        nc.sync.dma_start(out_d[:, :], out_sb[:])

    nc.compile()

    names = dict(prim="prim_t", w0p="w0p", w0x="w0x", w1x="w1x",
                 wout="wout_packed", bias0="bias0", bias1="bias1",
                 bout="bout", out="out")
    _CACHE["nc"] = nc
    _CACHE["names"] = names
    return nc, names


def _make_in_maps(primary_input, W0, b0, W1, b1, W_out, b_out):
    """Host-side sharding + layout prep (all cheap numpy except the
    feature-major transpose of the batch shards)."""
    primary_input = np.ascontiguousarray(primary_input, dtype=np.float32)
    W0 = np.asarray(W0, dtype=np.float32)
    b0 = np.asarray(b0, dtype=np.float32)
    W1 = np.asarray(W1, dtype=np.float32)
    b1 = np.asarray(b1, dtype=np.float32)
    W_out = np.asarray(W_out, dtype=np.float32)
    b_out = np.asarray(b_out, dtype=np.float32)

    import ml_dtypes
    ps = D_IN // T  # 1024
    w0p = np.ascontiguousarray(W0[:ps].astype(ml_dtypes.bfloat16))  # [1024, 512]
    w0x = np.ascontiguousarray(W0[ps:ps + F].astype(ml_dtypes.bfloat16))
    w0_last = W0[ps + F]                             # [512]
    w1x = np.ascontiguousarray(W1[:F].astype(ml_dtypes.bfloat16))
    w1_last = W1[F]                                  # [512]

    bias0 = np.concatenate(
        [(b0 + c * w0_last).reshape(FO, 128).T for c in range(NW)], axis=1)
    bias1 = np.concatenate(
        [(b1 + c * w1_last).reshape(FO, 128).T for c in range(NW)], axis=1)
    bias0 = np.ascontiguousarray(bias0, dtype=np.float32)   # [128, 16]
    bias1 = np.ascontiguousarray(bias1, dtype=np.float32)   # [128, 16]

    # wout_packed[p, k*10+o] = W_out[128k+p, o]
    wout_packed = np.ascontiguousarray(
        W_out.reshape(S, 128, N_OUT).transpose(1, 0, 2).reshape(128, S * N_OUT)
        .astype(ml_dtypes.bfloat16))
    bout = np.ascontiguousarray(b_out.reshape(N_OUT, 1))

    shared = dict(w0p=w0p, w0x=w0x, w1x=w1x, wout_packed=wout_packed,
                  bias0=bias0, bias1=bias1, bout=bout)
    in_maps = []
    for core in range(N_CORES):
        shard = primary_input[core * BC:(core + 1) * BC]          # [512, 8192]
        prim_t = np.ascontiguousarray(shard.T.astype(ml_dtypes.bfloat16))
        m = {"prim_t": prim_t}
        m.update(shared)
        in_maps.append(m)
    return in_maps


def _install_ntff_hook():
    """Provide antenv.axon_hooks (absent in this image) backed by ctypes
    calls into libaxon_pjrt.so, so run_bass_kernel_spmd(trace=True) can
    capture NTFF profiles. Mirrors trn_agent_boot.trn_boot."""
    import contextlib
    import ctypes
    import sys
    import types

    if "antenv.axon_hooks" in sys.modules:
        return
    so_path = "/opt/axon/libaxon_pjrt.so"
    lib = ctypes.CDLL(so_path)
    lib.axon_start_nrt_profile.argtypes = [ctypes.POINTER(ctypes.c_int64),
                                           ctypes.c_size_t]
    lib.axon_start_nrt_profile.restype = ctypes.c_int64
    lib.axon_stop_nrt_profile.argtypes = [ctypes.c_char_p]
    lib.axon_stop_nrt_profile.restype = ctypes.c_int64

    @contextlib.contextmanager
    def _hook(output_dir, device_ids):
        import jax
        jax.devices()
        if device_ids:
            ids = (ctypes.c_int64 * len(device_ids))(*device_ids)
            rc = lib.axon_start_nrt_profile(ids, len(device_ids))
        else:
            rc = lib.axon_start_nrt_profile(None, 0)
        if rc != 0:
            raise RuntimeError(f"axon_start_nrt_profile rc={rc}")
        try:
            yield
        finally:
            n = lib.axon_stop_nrt_profile(str(output_dir).encode())
            print(f"profile: {n} file(s) written to {output_dir}",
                  file=sys.stderr)

    mod = types.ModuleType("antenv.axon_hooks")
    mod.get_axon_ntff_profile_hook = lambda: _hook
    mod.set_axon_ntff_profile_hook = lambda h: None
    sys.modules["antenv.axon_hooks"] = mod
    import antenv
    antenv.axon_hooks = mod


def kernel(primary_input, W0, b0, W1, b1, W_out, b_out, _trace=False,
           _trace_cores=None):
    from concourse import bass_utils

    if _trace:
        _install_ntff_hook()

    nc, _ = _build_program()
    in_maps = _make_in_maps(primary_input, W0, b0, W1, b1, W_out, b_out)
    res = bass_utils.run_bass_kernel_spmd(
        nc, in_maps, core_ids=list(range(N_CORES)),
        trace=_trace, trace_cores=_trace_cores)
    out = np.empty((B_FULL, N_OUT), dtype=np.float32)
    for core in range(N_CORES):
        out[core * BC:(core + 1) * BC] = res.results[core]["out"].T
    if _trace:
        kernel._last_results = res
    return out



# revision 25
# speedup vs baseline: 1.0684x; 1.0650x over previous
"""Capsule-network kernel for 8x TRN2 NeuronCores (data-parallel over batch).

Reference computation (see problem):
  prim = primary_input.reshape(B, 8, 1024)
  prev = zeros(B, 4096)
  for col in 0..3:
    # layer0: inp = [prim_t, x_t, col] (1537) @ W0 -> relu -> flat -> roll(-128)
    # layer1: inp = [x_t, col] (513) @ W1 -> relu -> flat -> roll(+128)
  out = prev @ W_out + b_out

Kernel strategy (per core, batch shard Bc=512):
  - Everything on-chip is FEATURE-MAJOR: tiles are [128 features, Bc batch].
    ROLL=128 == partition count, so rolls are free tile re-indexings.
  - The scalar `col` concat input contributes col*W[last_row] to the
    pre-activation -> folded into per-col biases (computed on host).
  - P = prim @ W0[0:1024] is col-invariant -> computed once (phase 1),
    kept in SBUF as bf16, added during the layer0 drain each col.
  - col 0 layer0 has x=0 -> out = relu(P + b0): no matmuls at all.
  - Matmuls run as bf16 (same 1 col/cycle PE rate as fp32r, but
    the 2-byte LDWEIGHTS hides fully: measured cadence ~216ns vs
    ~227ns for fp32r).  Activations/weights bf16, psum fp32.
  - HW measurement: an fp32r matmul whose stationary weights differ from
    the previous matmul costs ~252ns; same-weights runs cost ~226.7ns.
    So layers are swept (fo, k) outer / row t inner: 8 consecutive
    matmuls share one weight tile (one sweep = 8 psum banks).
  - 6 dummy matmuls at t=0 (on a memset tile) ramp the PE out of its
    low p-state during the initial DMA wait.
"""

import numpy as np

# ---- problem constants (hardcoded; kernel.py must be self-contained) ----
B_FULL = 4096
D_IN = 8192
T = 8            # NUM_TALL
NW = 4           # NUM_WIDE
F = 512          # feature size per capsule row
ROLL = 128
N_CORES = 8
BC = B_FULL // N_CORES   # per-core batch = 512
S = (F * T) // 128       # state feature tiles = 32
KP = (D_IN // T) // 128  # prim k-tiles per capsule row = 8
KX = F // 128            # x k-tiles = 4
FO = F // 128            # output feature tiles per row-layer = 4
N_OUT = 10
N_WARM = 6               # dummy p-state warmup matmuls

_CACHE = {}


def _build_program():
    """Build (and cache) the single-core Bass program. Same program runs
    SPMD on all 8 cores with different batch shards."""
    if "nc" in _CACHE:
        return _CACHE["nc"], _CACHE["names"]

    from contextlib import ExitStack

    import concourse.tile as tile
    from concourse import bacc, mybir

    f32 = mybir.dt.float32
    f8 = mybir.dt.float8e4
    f32r = mybir.dt.float32r
    bf16 = mybir.dt.bfloat16
    AF = mybir.ActivationFunctionType
    DR = mybir.MatmulPerfMode.DoubleRow
    ADD = mybir.AluOpType.add

    nc = bacc.Bacc("TRN2", target_bir_lowering=False, debug=False,
                   num_devices=N_CORES)

    # prim + w0p travel and multiply as bf16: phase-1 is the only
    # DMA-heavy span (16.8MB of prim in fp32 saturates the 360 GB/s DMA
    # system during the cold start); halving the bytes costs ~26ns/matmul
    # of bf16 LDWEIGHTS overhead on the 256 P-matmuls but removes all
    # DMA-starvation stalls.
    prim_d = nc.dram_tensor("prim_t", [D_IN, BC], bf16, kind="ExternalInput").ap()
    w0p_d = nc.dram_tensor("w0p", [KP * 128, F], bf16, kind="ExternalInput").ap()
    w0x_d = nc.dram_tensor("w0x", [F, F], bf16, kind="ExternalInput").ap()
    w1x_d = nc.dram_tensor("w1x", [F, F], bf16, kind="ExternalInput").ap()
    wout_d = nc.dram_tensor("wout_packed", [128, S * N_OUT], bf16,
                            kind="ExternalInput").ap()
    w1x8_d = nc.dram_tensor("w1x8", [(KX // 2) * 128, 2, F], f8,
                            kind="ExternalInput").ap()
    bias016_d = nc.dram_tensor("bias0_16", [128, NW * FO], f32,
                               kind="ExternalInput").ap()
    bias0_d = nc.dram_tensor("bias0", [128, NW * FO], f32, kind="ExternalInput").ap()
    bias1_d = nc.dram_tensor("bias1", [128, NW * FO], f32, kind="ExternalInput").ap()
    bout_d = nc.dram_tensor("bout", [N_OUT, 1], f32, kind="ExternalInput").ap()
    out_d = nc.dram_tensor("out", [N_OUT, BC], f32, kind="ExternalOutput").ap()

    with tile.TileContext(nc) as tc, ExitStack() as ctx:
        const = ctx.enter_context(tc.tile_pool(name="const", bufs=1))
        state = ctx.enter_context(tc.tile_pool(name="state", bufs=1))
        cpool = ctx.enter_context(tc.tile_pool(name="cpool", bufs=33))
        prim_pool = ctx.enter_context(tc.tile_pool(name="primp", bufs=12))
        ppool = ctx.enter_context(tc.tile_pool(name="psum", bufs=8, space="PSUM"))

        # ---- constants ----
        w0p_sb = [const.tile([128, F], bf16, name=f"w0p{k}", tag=f"w0p{k}")
                  for k in range(KP)]
        w0x_sb = [const.tile([128, F], bf16, name=f"w0x{k}", tag=f"w0x{k}")
                  for k in range(KX)]
        w1x_sb = [const.tile([128, F], bf16, name=f"w1x{k}", tag=f"w1x{k}")
                  for k in range(KX)]
        wout_sb = const.tile([128, S * N_OUT], bf16, name="wout_sb", tag="wout")
        w1x8_sb = [const.tile([128, 2, F], f8, name=f"w1x8_{q}", tag=f"w1x8_{q}")
                   for q in range(KX // 2)]
        bias016_sb = const.tile([128, NW * FO], f32, name="bias016_sb",
                                tag="bias016")
        bias0_sb = const.tile([128, NW * FO], f32, name="bias0_sb", tag="bias0")
        bias1_sb = const.tile([128, NW * FO], f32, name="bias1_sb", tag="bias1")
        bout_sb = const.tile([N_OUT, 1], f32, name="bout_sb", tag="bout")
        warm_sb = const.tile([128, 128], f32, name="warm_sb", tag="warm")

        # ---- persistent state ----
        A = [state.tile([128, BC], bf16, name=f"state_a{i}", tag=f"A{i}")
             for i in range(S)]
        P = [state.tile([128, BC], bf16, name=f"state_p{i}", tag=f"P{i}")
             for i in range(S)]

        # ---- p-state warmup: dummy matmuls on a memset tile ----
        # (fp32 runs at 4 cycles/row so a 128-wide moving dim gives
        # ~213-790ns per dummy across the ramp)
        nc.vector.memset(warm_sb[:], 0.0)
        for i in range(N_WARM):
            ps = ppool.tile([128, BC], f32, name=f"warm{i}", tag="mm")
            nc.tensor.matmul(ps[0:128, 0:128], warm_sb[:], warm_sb[:],
                             start=True, stop=True)

        def load_deferred_consts(gi):
            # late-needed constants ride the idle gpsimd (SWDGE) queue so
            # the sync/scalar queues keep streaming prim.  w1x must be
            # resident by ~15us (first col0-layer1 chunk), so it goes out
            # at gi=0 -- the gpsimd queue has nothing else and issues it
            # at t~1us.
            if gi == 0:
                nc.sync.dma_start(bias0_sb[:], bias0_d[:, :])
                nc.gpsimd.dma_start(bias1_sb[:], bias1_d[:, :])
                nc.gpsimd.dma_start(bout_sb[:], bout_d[:, :])
                for k in range(KX):
                    nc.gpsimd.dma_start(w1x_sb[k][:], w1x_d[k * 128:(k + 1) * 128, :])
                for q in range(KX // 2):
                    nc.gpsimd.dma_start(w1x8_sb[q][:],
                                        w1x8_d[q * 128:(q + 1) * 128])
                nc.gpsimd.dma_start(bias016_sb[:], bias016_d[:, :])
            elif gi == 1:
                for k in range(KX):
                    nc.gpsimd.dma_start(w0x_sb[k][:], w0x_d[k * 128:(k + 1) * 128, :])
            elif gi == 5:
                nc.gpsimd.dma_start(wout_sb[:], wout_d[:, :])

        C = [None] * S  # col-current layer0 outputs (cpool ring tiles)

        def _pair_of(j):
            # fp8 DoubleRow pair p holds tiles (2p+1, 2p+2 % 32)
            return ((j - 1) % S) // 2, 1 - (j % 2)

        def layer1_fp8(c, C8p):
            # raw fp8 DoubleRow layer1: K=256 per instruction (2 chunks),
            # x = fp8(16*relu), W = fp8(32*W1x) -> psum scale 512.
            # Error is diluted by downstream relu/P mixing (measured).
            for fo in range(FO):
                pss = [ppool.tile([128, BC], f32, name=f"p8_{c}_{fo}_{t}",
                                  tag="mm") for t in range(T)]
                for q in range(KX // 2):
                    w_ap = w1x8_sb[q][:, :, fo * 128:(fo + 1) * 128]
                    for t in range(T):
                        nc.tensor.matmul(
                            pss[t][:], w_ap, C8p[(2 * t + q) % (S // 2)][:, :, :],
                            start=(q == 0), stop=(q == KX // 2 - 1),
                            perf_mode=DR)
                b1ap = bias1_sb[:, c * FO + fo:c * FO + fo + 1]
                for t in range(T):
                    # all-ACT: A = relu(psum/512 + b1c) needs scale+bias+max
                    nc.scalar.activation(A[t * FO + fo][:], pss[t][:], AF.Relu,
                                         bias=b1ap, scale=1.0 / 512)



        # ==== phase 1: P[j] = prim @ W0p (col-invariant), fused col0-L0 ====
        # Per-row groups (4 psum banks each): bf16 LDWEIGHTS hides fully
        # so weight-run batching is unnecessary, and small groups smooth
        # the psum-drain handoff between groups.
        # Drains: P copy (DVE, bf16) + col0-L0 C = relu(P+b0) (ACT).
        groups = [(t,) for t in range(T)]
        C8 = [cpool.tile([128, 2, BC], f8, name=f"c8a_{p}", tag="C")
              for p in range(S // 2)]

        def prim_dma(t, k):
            # one k-tile per DMA, full-width: the DMA system is
            # descriptor-line-rate limited, so 1KB lines (full bf16 rows)
            # move twice the bytes per line vs split halves.  Row 0 and
            # odd rows ride the scalar queue (it starts issuing ~5us
            # before sync, which carries the TileContext preamble).
            g = t * KP + k
            tile_ = prim_pool.tile([128, BC], bf16, name=f"prim_{g}",
                                   tag="prim")
            # scalar (ACT engine: also runs drains) only carries row 0,
            # interleaved with w0p.  Sync takes rows 1/2/4/6 (row 1 first,
            # right after bias0, so it lands before its ~14us deadline);
            # gpsimd takes rows 3/5/7 after the small deferred consts.
            if t == 0:
                q = nc.scalar
            elif t in (1, 2, 4, 6):
                q = nc.sync
            else:
                q = nc.gpsimd
            q.dma_start(tile_[:], prim_d[g * 128:(g + 1) * 128, :])
            return tile_

        def layer1_chunk(c, rows):
            # layer1 for a subset of rows (weight run-of-len(rows)).
            # A[4t+fo] = relu(W1x.T C + b1c);  C k-tile = C[(4t+k+1)%S]
            for fo in range(FO):
                pss = {t: ppool.tile([128, BC], f32, name=f"ps1_{c}_{fo}_{t}",
                                     tag="mm") for t in rows}
                for k in range(KX):
                    w_ap = w1x_sb[k][:, fo * 128:(fo + 1) * 128]
                    for t in rows:
                        nc.tensor.matmul(
                            pss[t][:], w_ap, C[(t * FO + k + 1) % S][:],
                            start=(k == 0), stop=(k == KX - 1))
                b1ap = bias1_sb[:, c * FO + fo:c * FO + fo + 1]
                for t in rows:
                    j = t * FO + fo
                    if t % 2 == 0:
                        nc.scalar.activation(A[j][:], pss[t][:], AF.Relu,
                                             bias=b1ap)
                    else:
                        # relu(psum + bias) on DVE: (psum add bias) max 0
                        nc.vector.tensor_scalar(A[j][:], pss[t][:], b1ap, 0.0,
                                                ADD, mybir.AluOpType.max)

        for gi, grp in enumerate(groups):
            pss = {}
            for t in grp:
                for fo in range(FO):
                    pss[(t, fo)] = ppool.tile([128, BC], f32,
                                              name=f"ps_p1_{t}_{fo}", tag="mm")
            pt = {}
            if gi == 0:
                # interleave w0p with row 0's tiles on scalar so the
                # k-th matmul's pair (w0p[k], prim(0,k)) lands together
                for k in range(KP):
                    nc.scalar.dma_start(w0p_sb[k][:],
                                        w0p_d[k * 128:(k + 1) * 128, :])
                    pt[(0, k)] = prim_dma(0, k)
            else:
                for k in range(KP):
                    for t in grp:
                        pt[(t, k)] = prim_dma(t, k)
            for k in range(KP):
                for fo in range(FO):
                    for t in grp:
                        nc.tensor.matmul(
                            pss[(t, fo)][:],
                            w0p_sb[k][:, fo * 128:(fo + 1) * 128],
                            pt[(t, k)][:],
                            start=(k == 0), stop=(k == KP - 1))
            load_deferred_consts(gi)
            for t in grp:
                for fo in range(FO):
                    j = t * FO + fo
                    nc.vector.tensor_copy(P[j][:], pss[(t, fo)][:])
                    # col0-L1 runs raw fp8 DoubleRow: C stored as fp8
                    # pair-tile halves at scale 16
                    p, h = _pair_of(j)
                    nc.scalar.activation(C8[p][:, h, :], pss[(t, fo)][:],
                                         AF.Relu,
                                         bias=bias016_sb[:, fo:fo + 1],
                                         scale=16.0)
        # col-0 layer1 with full run-of-8 weight reuse (bf16 phase-1 DMA
        # leaves enough bandwidth slack that no absorber work is needed)
        layer1_fp8(0, C8)

        # ==== layer emitters: (fo, k) outer, t inner -> weight run-of-8 ====
        def layer0_col(c, out8=None):
            # C[4t+fo] = relu(W0x.T x + P + b0c);  x k-tile = A[(4t+k-1)%S]
            # The t sweep starts at t=1: the k=0 input A[4t-1] is a fo3
            # tile of the previous col's layer1, and t=0 needs A[31] --
            # the very LAST drain of that col.  Rotating gives each A
            # one extra sweep-step of drain slack.
            rows = [(1 + i) % T for i in range(T)]
            for fo in range(FO):
                pss = {t: ppool.tile([128, BC], f32, name=f"ps0_{c}_{fo}_{t}",
                                     tag="mm") for t in rows}
                for k in range(KX):
                    w_ap = w0x_sb[k][:, fo * 128:(fo + 1) * 128]
                    for t in rows:
                        nc.tensor.matmul(
                            pss[t][:], w_ap, A[(t * FO + k - 1) % S][:],
                            start=(k == 0), stop=(k == KX - 1))
                b0ap = bias0_sb[:, c * FO + fo:c * FO + fo + 1]
                for t in rows:
                    j = t * FO + fo
                    ct = cpool.tile([128, BC], bf16, name=f"c{c}_{j}", tag="C")
                    # ct = (psum + bias0_c) + P  on DVE, then relu on ACT
                    nc.vector.scalar_tensor_tensor(
                        ct[:], pss[t][:], b0ap, P[j][:], ADD, ADD)
                    if out8 is None:
                        nc.scalar.activation(ct[:], ct[:], AF.Relu)
                        C[j] = ct
                    else:
                        # next layer1 is fp8: C8 half = fp8(16*relu(ct))
                        p, h = _pair_of(j)
                        nc.scalar.activation(out8[p][:, h, :], ct[:], AF.Relu,
                                             scale=16.0)

        def layer1_col(c):
            # A[4t+fo] = relu(W1x.T C + b1c);  C k-tile = C[(4t+k+1)%S]
            for fo in range(FO):
                pss = [ppool.tile([128, BC], f32, name=f"ps1_{c}_{fo}_{t}",
                                  tag="mm") for t in range(T)]
                for k in range(KX):
                    w_ap = w1x_sb[k][:, fo * 128:(fo + 1) * 128]
                    for t in range(T):
                        nc.tensor.matmul(
                            pss[t][:], w_ap, C[(t * FO + k + 1) % S][:],
                            start=(k == 0), stop=(k == KX - 1))
                b1ap = bias1_sb[:, c * FO + fo:c * FO + fo + 1]
                for t in range(T):
                    j = t * FO + fo
                    if t % 2 == 0:
                        nc.scalar.activation(A[j][:], pss[t][:], AF.Relu,
                                             bias=b1ap)
                    else:
                        # relu(psum + bias) on DVE: (psum add bias) max 0
                        nc.vector.tensor_scalar(A[j][:], pss[t][:], b1ap, 0.0,
                                                ADD, mybir.AluOpType.max)

        # ==== cols 1..3 (col-0 layer1 was interleaved into phase 1) ====
        C8b = [cpool.tile([128, 2, BC], f8, name=f"c8b_{p}", tag="C")
               for p in range(S // 2)]
        layer0_col(1, out8=C8b)
        layer1_fp8(1, C8b)
        for c in range(2, NW):
            layer0_col(c)
            layer1_col(c)

        # ---- final: out = prev @ W_out + b_out;  prev[k] = A[(k-1) % S] ----
        psf_full = ppool.tile([128, BC], f32, name="psf", tag="mm")
        psf = psf_full[0:N_OUT, :]
        # emit in col-3's A-write order (sweep fo, then t) so the
        # accumulation chain chases the layer1 drains
        n = 0
        for fo in range(FO):
            for t in range(T):
                k = (t * FO + fo + 1) % S
                nc.tensor.matmul(
                    psf[:],
                    wout_sb[:, k * N_OUT:(k + 1) * N_OUT],
                    A[(k - 1) % S][:],
                    start=(n == 0), stop=(n == S - 1))
                n += 1
        out_sb = cpool.tile([N_OUT, BC], f32, name="out_sb", tag="C")
        nc.scalar.activation(out_sb[:], psf[:], AF.Identity, bias=bout_sb[:])
        nc.sync.dma_start(out_d[:, :], out_sb[:])

    nc.compile()

    names = dict(prim="prim_t", w0p="w0p", w0x="w0x", w1x="w1x",
                 wout="wout_packed", bias0="bias0", bias1="bias1",
                 bout="bout", out="out")
    _CACHE["nc"] = nc
    _CACHE["names"] = names
    return nc, names


def _make_in_maps(primary_input, W0, b0, W1, b1, W_out, b_out):
    """Host-side sharding + layout prep (all cheap numpy except the
    feature-major transpose of the batch shards)."""
    primary_input = np.ascontiguousarray(primary_input, dtype=np.float32)
    W0 = np.asarray(W0, dtype=np.float32)
    b0 = np.asarray(b0, dtype=np.float32)
    W1 = np.asarray(W1, dtype=np.float32)
    b1 = np.asarray(b1, dtype=np.float32)
    W_out = np.asarray(W_out, dtype=np.float32)
    b_out = np.asarray(b_out, dtype=np.float32)

    import ml_dtypes
    F8NP = ml_dtypes.float8_e4m3
    ps = D_IN // T  # 1024
    w0p = np.ascontiguousarray(W0[:ps].astype(ml_dtypes.bfloat16))  # [1024, 512]
    w0x = np.ascontiguousarray(W0[ps:ps + F].astype(ml_dtypes.bfloat16))
    w0_last = W0[ps + F]                             # [512]
    w1x = np.ascontiguousarray(W1[:F].astype(ml_dtypes.bfloat16))
    w1_last = W1[F]                                  # [512]

    bias0 = np.concatenate(
        [(b0 + c * w0_last).reshape(FO, 128).T for c in range(NW)], axis=1)
    bias1 = np.concatenate(
        [(b1 + c * w1_last).reshape(FO, 128).T for c in range(NW)], axis=1)
    bias0 = np.ascontiguousarray(bias0, dtype=np.float32)   # [128, 16]
    bias1 = np.ascontiguousarray(bias1, dtype=np.float32)   # [128, 16]

    # wout_packed[p, k*10+o] = W_out[128k+p, o]
    wout_packed = np.ascontiguousarray(
        W_out.reshape(S, 128, N_OUT).transpose(1, 0, 2).reshape(128, S * N_OUT)
        .astype(ml_dtypes.bfloat16))
    bout = np.ascontiguousarray(b_out.reshape(N_OUT, 1))

    # fp8 layer1 weights: pair-packed DoubleRow chunks of 32*W1x
    w1f = (32.0 * np.asarray(W1[:F], dtype=np.float32)).astype(F8NP)
    w1x8 = np.ascontiguousarray(
        w1f.reshape(KX // 2, 2, 128, F).transpose(0, 2, 1, 3)
        .reshape((KX // 2) * 128, 2, F))
    bias0_16 = np.ascontiguousarray(16.0 * bias0)
    shared = dict(w0p=w0p, w0x=w0x, w1x=w1x, wout_packed=wout_packed,
                  bias0=bias0, bias1=bias1, bout=bout,
                  w1x8=w1x8, bias0_16=bias0_16)
    in_maps = []
    for core in range(N_CORES):
        shard = primary_input[core * BC:(core + 1) * BC]          # [512, 8192]
        prim_t = np.ascontiguousarray(shard.T.astype(ml_dtypes.bfloat16))
        m = {"prim_t": prim_t}
        m.update(shared)
        in_maps.append(m)
    return in_maps


def _install_ntff_hook():
    """Provide antenv.axon_hooks (absent in this image) backed by ctypes
    calls into libaxon_pjrt.so, so run_bass_kernel_spmd(trace=True) can
    capture NTFF profiles. Mirrors trn_agent_boot.trn_boot."""
    import contextlib
    import ctypes
    import sys
    import types

    if "antenv.axon_hooks" in sys.modules:
        return
    so_path = "/opt/axon/libaxon_pjrt.so"
    lib = ctypes.CDLL(so_path)
    lib.axon_start_nrt_profile.argtypes = [ctypes.POINTER(ctypes.c_int64),
                                           ctypes.c_size_t]
    lib.axon_start_nrt_profile.restype = ctypes.c_int64
    lib.axon_stop_nrt_profile.argtypes = [ctypes.c_char_p]
    lib.axon_stop_nrt_profile.restype = ctypes.c_int64

    @contextlib.contextmanager
    def _hook(output_dir, device_ids):
        import jax
        jax.devices()
        if device_ids:
            ids = (ctypes.c_int64 * len(device_ids))(*device_ids)
            rc = lib.axon_start_nrt_profile(ids, len(device_ids))
        else:
            rc = lib.axon_start_nrt_profile(None, 0)
        if rc != 0:
            raise RuntimeError(f"axon_start_nrt_profile rc={rc}")
        try:
            yield
        finally:
            n = lib.axon_stop_nrt_profile(str(output_dir).encode())
            print(f"profile: {n} file(s) written to {output_dir}",
                  file=sys.stderr)

    mod = types.ModuleType("antenv.axon_hooks")
    mod.get_axon_ntff_profile_hook = lambda: _hook
    mod.set_axon_ntff_profile_hook = lambda h: None
    sys.modules["antenv.axon_hooks"] = mod
    import antenv
    antenv.axon_hooks = mod


def kernel(primary_input, W0, b0, W1, b1, W_out, b_out, _trace=False,
           _trace_cores=None):
    from concourse import bass_utils

    if _trace:
        _install_ntff_hook()

    nc, _ = _build_program()
    in_maps = _make_in_maps(primary_input, W0, b0, W1, b1, W_out, b_out)
    res = bass_utils.run_bass_kernel_spmd(
        nc, in_maps, core_ids=list(range(N_CORES)),
        trace=_trace, trace_cores=_trace_cores)
    out = np.empty((B_FULL, N_OUT), dtype=np.float32)
    for core in range(N_CORES):
        out[core * BC:(core + 1) * BC] = res.results[core]["out"].T
    if _trace:
        kernel._last_results = res
    return out



# revision 26
# speedup vs baseline: 1.1028x; 1.0322x over previous
"""Capsule-network kernel for 8x TRN2 NeuronCores (data-parallel over batch).

Reference computation (see problem):
  prim = primary_input.reshape(B, 8, 1024)
  prev = zeros(B, 4096)
  for col in 0..3:
    # layer0: inp = [prim_t, x_t, col] (1537) @ W0 -> relu -> flat -> roll(-128)
    # layer1: inp = [x_t, col] (513) @ W1 -> relu -> flat -> roll(+128)
  out = prev @ W_out + b_out

Kernel strategy (per core, batch shard Bc=512):
  - Everything on-chip is FEATURE-MAJOR: tiles are [128 features, Bc batch].
    ROLL=128 == partition count, so rolls are free tile re-indexings.
  - The scalar `col` concat input contributes col*W[last_row] to the
    pre-activation -> folded into per-col biases (computed on host).
  - P = prim @ W0[0:1024] is col-invariant -> computed once (phase 1),
    kept in SBUF as bf16, added during the layer0 drain each col.
  - col 0 layer0 has x=0 -> out = relu(P + b0): no matmuls at all.
  - Matmuls run as bf16 (same 1 col/cycle PE rate as fp32r, but
    the 2-byte LDWEIGHTS hides fully: measured cadence ~216ns vs
    ~227ns for fp32r).  Activations/weights bf16, psum fp32.
  - HW measurement: an fp32r matmul whose stationary weights differ from
    the previous matmul costs ~252ns; same-weights runs cost ~226.7ns.
    So layers are swept (fo, k) outer / row t inner: 8 consecutive
    matmuls share one weight tile (one sweep = 8 psum banks).
  - 6 dummy matmuls at t=0 (on a memset tile) ramp the PE out of its
    low p-state during the initial DMA wait.
"""

import numpy as np

# ---- problem constants (hardcoded; kernel.py must be self-contained) ----
B_FULL = 4096
D_IN = 8192
T = 8            # NUM_TALL
NW = 4           # NUM_WIDE
F = 512          # feature size per capsule row
ROLL = 128
N_CORES = 8
BC = B_FULL // N_CORES   # per-core batch = 512
S = (F * T) // 128       # state feature tiles = 32
KP = (D_IN // T) // 128  # prim k-tiles per capsule row = 8
KX = F // 128            # x k-tiles = 4
FO = F // 128            # output feature tiles per row-layer = 4
N_OUT = 10
N_WARM = 6               # dummy p-state warmup matmuls

_CACHE = {}


def _build_program():
    """Build (and cache) the single-core Bass program. Same program runs
    SPMD on all 8 cores with different batch shards."""
    if "nc" in _CACHE:
        return _CACHE["nc"], _CACHE["names"]

    from contextlib import ExitStack

    import concourse.tile as tile
    from concourse import bacc, mybir

    f32 = mybir.dt.float32
    f8 = mybir.dt.float8e4
    f32r = mybir.dt.float32r
    bf16 = mybir.dt.bfloat16
    AF = mybir.ActivationFunctionType
    DR = mybir.MatmulPerfMode.DoubleRow
    ADD = mybir.AluOpType.add

    nc = bacc.Bacc("TRN2", target_bir_lowering=False, debug=False,
                   num_devices=N_CORES)

    # prim + w0p travel and multiply as bf16: phase-1 is the only
    # DMA-heavy span (16.8MB of prim in fp32 saturates the 360 GB/s DMA
    # system during the cold start); halving the bytes costs ~26ns/matmul
    # of bf16 LDWEIGHTS overhead on the 256 P-matmuls but removes all
    # DMA-starvation stalls.
    prim_d = nc.dram_tensor("prim_t", [D_IN, BC], bf16, kind="ExternalInput").ap()
    w0p_d = nc.dram_tensor("w0p", [KP * 128, F], bf16, kind="ExternalInput").ap()
    w0x_d = nc.dram_tensor("w0x", [F, F], bf16, kind="ExternalInput").ap()
    w1x_d = nc.dram_tensor("w1x", [F, F], bf16, kind="ExternalInput").ap()
    wout_d = nc.dram_tensor("wout_packed", [128, S * N_OUT], bf16,
                            kind="ExternalInput").ap()
    w1x8_d = nc.dram_tensor("w1x8", [(KX // 2) * 128, 2, F], f8,
                            kind="ExternalInput").ap()
    bias016_d = nc.dram_tensor("bias0_16", [128, NW * FO], f32,
                               kind="ExternalInput").ap()
    bias0_d = nc.dram_tensor("bias0", [128, NW * FO], f32, kind="ExternalInput").ap()
    bias1_d = nc.dram_tensor("bias1", [128, NW * FO], f32, kind="ExternalInput").ap()
    bout_d = nc.dram_tensor("bout", [N_OUT, 1], f32, kind="ExternalInput").ap()
    out_d = nc.dram_tensor("out", [N_OUT, BC], f32, kind="ExternalOutput").ap()

    with tile.TileContext(nc) as tc, ExitStack() as ctx:
        const = ctx.enter_context(tc.tile_pool(name="const", bufs=1))
        state = ctx.enter_context(tc.tile_pool(name="state", bufs=1))
        cpool = ctx.enter_context(tc.tile_pool(name="cpool", bufs=33))
        prim_pool = ctx.enter_context(tc.tile_pool(name="primp", bufs=12))
        ppool = ctx.enter_context(tc.tile_pool(name="psum", bufs=8, space="PSUM"))

        # ---- constants ----
        w0p_sb = [const.tile([128, F], bf16, name=f"w0p{k}", tag=f"w0p{k}")
                  for k in range(KP)]
        w0x_sb = [const.tile([128, F], bf16, name=f"w0x{k}", tag=f"w0x{k}")
                  for k in range(KX)]
        w1x_sb = [const.tile([128, F], bf16, name=f"w1x{k}", tag=f"w1x{k}")
                  for k in range(KX)]
        wout_sb = const.tile([128, S * N_OUT], bf16, name="wout_sb", tag="wout")
        w1x8_sb = [const.tile([128, 2, F], f8, name=f"w1x8_{q}", tag=f"w1x8_{q}")
                   for q in range(KX // 2)]
        bias016_sb = const.tile([128, NW * FO], f32, name="bias016_sb",
                                tag="bias016")
        bias0_sb = const.tile([128, NW * FO], f32, name="bias0_sb", tag="bias0")
        bias1_sb = const.tile([128, NW * FO], f32, name="bias1_sb", tag="bias1")
        bout_sb = const.tile([N_OUT, 1], f32, name="bout_sb", tag="bout")
        warm_sb = const.tile([128, 128], f32, name="warm_sb", tag="warm")

        # ---- persistent state ----
        A = [state.tile([128, BC], bf16, name=f"state_a{i}", tag=f"A{i}")
             for i in range(S)]
        P = [state.tile([128, BC], bf16, name=f"state_p{i}", tag=f"P{i}")
             for i in range(S)]

        # ---- p-state warmup: dummy matmuls on a memset tile ----
        # (fp32 runs at 4 cycles/row so a 128-wide moving dim gives
        # ~213-790ns per dummy across the ramp)
        nc.vector.memset(warm_sb[:], 0.0)
        for i in range(N_WARM):
            ps = ppool.tile([128, BC], f32, name=f"warm{i}", tag="mm")
            nc.tensor.matmul(ps[0:128, 0:128], warm_sb[:], warm_sb[:],
                             start=True, stop=True)

        def load_deferred_consts(gi):
            # late-needed constants ride the idle gpsimd (SWDGE) queue so
            # the sync/scalar queues keep streaming prim.  w1x must be
            # resident by ~15us (first col0-layer1 chunk), so it goes out
            # at gi=0 -- the gpsimd queue has nothing else and issues it
            # at t~1us.
            if gi == 0:
                nc.sync.dma_start(bias0_sb[:], bias0_d[:, :])
                nc.gpsimd.dma_start(bias1_sb[:], bias1_d[:, :])
                nc.gpsimd.dma_start(bout_sb[:], bout_d[:, :])
                for k in range(KX):
                    nc.gpsimd.dma_start(w1x_sb[k][:], w1x_d[k * 128:(k + 1) * 128, :])
                for q in range(KX // 2):
                    nc.gpsimd.dma_start(w1x8_sb[q][:],
                                        w1x8_d[q * 128:(q + 1) * 128])
                nc.gpsimd.dma_start(bias016_sb[:], bias016_d[:, :])
            elif gi == 1:
                for k in range(KX):
                    nc.gpsimd.dma_start(w0x_sb[k][:], w0x_d[k * 128:(k + 1) * 128, :])
            elif gi == 5:
                nc.gpsimd.dma_start(wout_sb[:], wout_d[:, :])

        C = [None] * S  # col-current layer0 outputs (cpool ring tiles)

        def _pair_of(j):
            # fp8 DoubleRow pair p holds tiles (2p+1, 2p+2 % 32)
            return ((j - 1) % S) // 2, 1 - (j % 2)

        def layer1_fp8(c, C8p):
            # raw fp8 DoubleRow layer1: K=256 per instruction (2 chunks),
            # x = fp8(16*relu), W = fp8(32*W1x) -> psum scale 512.
            # Error is diluted by downstream relu/P mixing (measured).
            for fo in range(FO):
                pss = [ppool.tile([128, BC], f32, name=f"p8_{c}_{fo}_{t}",
                                  tag="mm") for t in range(T)]
                for q in range(KX // 2):
                    w_ap = w1x8_sb[q][:, :, fo * 128:(fo + 1) * 128]
                    for t in range(T):
                        nc.tensor.matmul(
                            pss[t][:], w_ap, C8p[(2 * t + q) % (S // 2)][:, :, :],
                            start=(q == 0), stop=(q == KX // 2 - 1),
                            perf_mode=DR)
                b1ap = bias1_sb[:, c * FO + fo:c * FO + fo + 1]
                for t in range(T):
                    # all-ACT: A = relu(psum/512 + b1c) needs scale+bias+max
                    nc.scalar.activation(A[t * FO + fo][:], pss[t][:], AF.Relu,
                                         bias=b1ap, scale=1.0 / 512)



        # ==== phase 1: P[j] = prim @ W0p (col-invariant), fused col0-L0 ====
        # Per-row groups (4 psum banks each): bf16 LDWEIGHTS hides fully
        # so weight-run batching is unnecessary, and small groups smooth
        # the psum-drain handoff between groups.
        # Drains: P copy (DVE, bf16) + col0-L0 C = relu(P+b0) (ACT).
        groups = [(t,) for t in range(T)]
        C8 = [cpool.tile([128, 2, BC], f8, name=f"c8a_{p}", tag="C")
              for p in range(S // 2)]

        def prim_dma(t, k):
            # one k-tile per DMA, full-width: the DMA system is
            # descriptor-line-rate limited, so 1KB lines (full bf16 rows)
            # move twice the bytes per line vs split halves.  Row 0 and
            # odd rows ride the scalar queue (it starts issuing ~5us
            # before sync, which carries the TileContext preamble).
            g = t * KP + k
            tile_ = prim_pool.tile([128, BC], bf16, name=f"prim_{g}",
                                   tag="prim")
            # scalar (ACT engine: also runs drains) only carries row 0,
            # interleaved with w0p.  Sync takes rows 1/2/4/6 (row 1 first,
            # right after bias0, so it lands before its ~14us deadline);
            # gpsimd takes rows 3/5/7 after the small deferred consts.
            if t == 0:
                q = nc.scalar
            elif t in (1, 2, 4, 6):
                q = nc.sync
            else:
                q = nc.gpsimd
            q.dma_start(tile_[:], prim_d[g * 128:(g + 1) * 128, :])
            return tile_

        def layer1_chunk(c, rows):
            # layer1 for a subset of rows (weight run-of-len(rows)).
            # A[4t+fo] = relu(W1x.T C + b1c);  C k-tile = C[(4t+k+1)%S]
            for fo in range(FO):
                pss = {t: ppool.tile([128, BC], f32, name=f"ps1_{c}_{fo}_{t}",
                                     tag="mm") for t in rows}
                for k in range(KX):
                    w_ap = w1x_sb[k][:, fo * 128:(fo + 1) * 128]
                    for t in rows:
                        nc.tensor.matmul(
                            pss[t][:], w_ap, C[(t * FO + k + 1) % S][:],
                            start=(k == 0), stop=(k == KX - 1))
                b1ap = bias1_sb[:, c * FO + fo:c * FO + fo + 1]
                for t in rows:
                    j = t * FO + fo
                    if t % 2 == 0:
                        nc.scalar.activation(A[j][:], pss[t][:], AF.Relu,
                                             bias=b1ap)
                    else:
                        # relu(psum + bias) on DVE: (psum add bias) max 0
                        nc.vector.tensor_scalar(A[j][:], pss[t][:], b1ap, 0.0,
                                                ADD, mybir.AluOpType.max)

        for gi, grp in enumerate(groups):
            pss = {}
            for t in grp:
                for fo in range(FO):
                    pss[(t, fo)] = ppool.tile([128, BC], f32,
                                              name=f"ps_p1_{t}_{fo}", tag="mm")
            pt = {}
            if gi == 0:
                # interleave w0p with row 0's tiles on scalar so the
                # k-th matmul's pair (w0p[k], prim(0,k)) lands together
                for k in range(KP):
                    nc.scalar.dma_start(w0p_sb[k][:],
                                        w0p_d[k * 128:(k + 1) * 128, :])
                    pt[(0, k)] = prim_dma(0, k)
            else:
                for k in range(KP):
                    for t in grp:
                        pt[(t, k)] = prim_dma(t, k)
            for k in range(KP):
                for fo in range(FO):
                    for t in grp:
                        nc.tensor.matmul(
                            pss[(t, fo)][:],
                            w0p_sb[k][:, fo * 128:(fo + 1) * 128],
                            pt[(t, k)][:],
                            start=(k == 0), stop=(k == KP - 1))
            load_deferred_consts(gi)
            for t in grp:
                for fo in range(FO):
                    j = t * FO + fo
                    nc.vector.tensor_copy(P[j][:], pss[(t, fo)][:])
                    # col0-L1 runs raw fp8 DoubleRow: C stored as fp8
                    # pair-tile halves at scale 16
                    p, h = _pair_of(j)
                    nc.scalar.activation(C8[p][:, h, :], pss[(t, fo)][:],
                                         AF.Relu,
                                         bias=bias016_sb[:, fo:fo + 1],
                                         scale=16.0)
        # col-0 layer1 with full run-of-8 weight reuse (bf16 phase-1 DMA
        # leaves enough bandwidth slack that no absorber work is needed)
        layer1_fp8(0, C8)

        # ==== layer emitters: (fo, k) outer, t inner -> weight run-of-8 ====
        def layer0_col(c, out8=None):
            # C[4t+fo] = relu(W0x.T x + P + b0c);  x k-tile = A[(4t+k-1)%S]
            # The t sweep starts at t=1: the k=0 input A[4t-1] is a fo3
            # tile of the previous col's layer1, and t=0 needs A[31] --
            # the very LAST drain of that col.  Rotating gives each A
            # one extra sweep-step of drain slack.
            rows = [(1 + i) % T for i in range(T)]
            for fo in range(FO):
                pss = {t: ppool.tile([128, BC], f32, name=f"ps0_{c}_{fo}_{t}",
                                     tag="mm") for t in rows}
                for k in range(KX):
                    w_ap = w0x_sb[k][:, fo * 128:(fo + 1) * 128]
                    for t in rows:
                        nc.tensor.matmul(
                            pss[t][:], w_ap, A[(t * FO + k - 1) % S][:],
                            start=(k == 0), stop=(k == KX - 1))
                b0ap = bias0_sb[:, c * FO + fo:c * FO + fo + 1]
                for t in rows:
                    j = t * FO + fo
                    ct = cpool.tile([128, BC], bf16, name=f"c{c}_{j}", tag="C")
                    # ct = (psum + bias0_c) + P  on DVE, then relu on ACT
                    nc.vector.scalar_tensor_tensor(
                        ct[:], pss[t][:], b0ap, P[j][:], ADD, ADD)
                    if out8 is None:
                        nc.scalar.activation(ct[:], ct[:], AF.Relu)
                        C[j] = ct
                    else:
                        # next layer1 is fp8: C8 half = fp8(16*relu(ct))
                        p, h = _pair_of(j)
                        nc.scalar.activation(out8[p][:, h, :], ct[:], AF.Relu,
                                             scale=16.0)

        def layer1_col(c):
            # A[4t+fo] = relu(W1x.T C + b1c);  C k-tile = C[(4t+k+1)%S]
            for fo in range(FO):
                pss = [ppool.tile([128, BC], f32, name=f"ps1_{c}_{fo}_{t}",
                                  tag="mm") for t in range(T)]
                for k in range(KX):
                    w_ap = w1x_sb[k][:, fo * 128:(fo + 1) * 128]
                    for t in range(T):
                        nc.tensor.matmul(
                            pss[t][:], w_ap, C[(t * FO + k + 1) % S][:],
                            start=(k == 0), stop=(k == KX - 1))
                b1ap = bias1_sb[:, c * FO + fo:c * FO + fo + 1]
                for t in range(T):
                    j = t * FO + fo
                    if t % 2 == 0:
                        nc.scalar.activation(A[j][:], pss[t][:], AF.Relu,
                                             bias=b1ap)
                    else:
                        # relu(psum + bias) on DVE: (psum add bias) max 0
                        nc.vector.tensor_scalar(A[j][:], pss[t][:], b1ap, 0.0,
                                                ADD, mybir.AluOpType.max)

        # ==== cols 1..3 (col-0 layer1 was interleaved into phase 1) ====
        C8b = [cpool.tile([128, 2, BC], f8, name=f"c8b_{p}", tag="C")
               for p in range(S // 2)]
        layer0_col(1, out8=C8b)
        layer1_fp8(1, C8b)
        C8c = [cpool.tile([128, 2, BC], f8, name=f"c8c_{p}", tag="C")
               for p in range(S // 2)]
        layer0_col(2, out8=C8c)
        layer1_fp8(2, C8c)
        layer0_col(3)
        layer1_col(3)

        # ---- final: out = prev @ W_out + b_out;  prev[k] = A[(k-1) % S] ----
        psf_full = ppool.tile([128, BC], f32, name="psf", tag="mm")
        psf = psf_full[0:N_OUT, :]
        # emit in col-3's A-write order (sweep fo, then t) so the
        # accumulation chain chases the layer1 drains
        n = 0
        for fo in range(FO):
            for t in range(T):
                k = (t * FO + fo + 1) % S
                nc.tensor.matmul(
                    psf[:],
                    wout_sb[:, k * N_OUT:(k + 1) * N_OUT],
                    A[(k - 1) % S][:],
                    start=(n == 0), stop=(n == S - 1))
                n += 1
        out_sb = cpool.tile([N_OUT, BC], f32, name="out_sb", tag="C")
        nc.scalar.activation(out_sb[:], psf[:], AF.Identity, bias=bout_sb[:])
        nc.sync.dma_start(out_d[:, :], out_sb[:])

    nc.compile()

    names = dict(prim="prim_t", w0p="w0p", w0x="w0x", w1x="w1x",
                 wout="wout_packed", bias0="bias0", bias1="bias1",
                 bout="bout", out="out")
    _CACHE["nc"] = nc
    _CACHE["names"] = names
    return nc, names


def _make_in_maps(primary_input, W0, b0, W1, b1, W_out, b_out):
    """Host-side sharding + layout prep (all cheap numpy except the
    feature-major transpose of the batch shards)."""
    primary_input = np.ascontiguousarray(primary_input, dtype=np.float32)
    W0 = np.asarray(W0, dtype=np.float32)
    b0 = np.asarray(b0, dtype=np.float32)
    W1 = np.asarray(W1, dtype=np.float32)
    b1 = np.asarray(b1, dtype=np.float32)
    W_out = np.asarray(W_out, dtype=np.float32)
    b_out = np.asarray(b_out, dtype=np.float32)

    import ml_dtypes
    F8NP = ml_dtypes.float8_e4m3
    ps = D_IN // T  # 1024
    w0p = np.ascontiguousarray(W0[:ps].astype(ml_dtypes.bfloat16))  # [1024, 512]
    w0x = np.ascontiguousarray(W0[ps:ps + F].astype(ml_dtypes.bfloat16))
    w0_last = W0[ps + F]                             # [512]
    w1x = np.ascontiguousarray(W1[:F].astype(ml_dtypes.bfloat16))
    w1_last = W1[F]                                  # [512]

    bias0 = np.concatenate(
        [(b0 + c * w0_last).reshape(FO, 128).T for c in range(NW)], axis=1)
    bias1 = np.concatenate(
        [(b1 + c * w1_last).reshape(FO, 128).T for c in range(NW)], axis=1)
    bias0 = np.ascontiguousarray(bias0, dtype=np.float32)   # [128, 16]
    bias1 = np.ascontiguousarray(bias1, dtype=np.float32)   # [128, 16]

    # wout_packed[p, k*10+o] = W_out[128k+p, o]
    wout_packed = np.ascontiguousarray(
        W_out.reshape(S, 128, N_OUT).transpose(1, 0, 2).reshape(128, S * N_OUT)
        .astype(ml_dtypes.bfloat16))
    bout = np.ascontiguousarray(b_out.reshape(N_OUT, 1))

    # fp8 layer1 weights: pair-packed DoubleRow chunks of 32*W1x
    w1f = (32.0 * np.asarray(W1[:F], dtype=np.float32)).astype(F8NP)
    w1x8 = np.ascontiguousarray(
        w1f.reshape(KX // 2, 2, 128, F).transpose(0, 2, 1, 3)
        .reshape((KX // 2) * 128, 2, F))
    bias0_16 = np.ascontiguousarray(16.0 * bias0)
    shared = dict(w0p=w0p, w0x=w0x, w1x=w1x, wout_packed=wout_packed,
                  bias0=bias0, bias1=bias1, bout=bout,
                  w1x8=w1x8, bias0_16=bias0_16)
    in_maps = []
    for core in range(N_CORES):
        shard = primary_input[core * BC:(core + 1) * BC]          # [512, 8192]
        prim_t = np.ascontiguousarray(shard.T.astype(ml_dtypes.bfloat16))
        m = {"prim_t": prim_t}
        m.update(shared)
        in_maps.append(m)
    return in_maps


def _install_ntff_hook():
    """Provide antenv.axon_hooks (absent in this image) backed by ctypes
    calls into libaxon_pjrt.so, so run_bass_kernel_spmd(trace=True) can
    capture NTFF profiles. Mirrors trn_agent_boot.trn_boot."""
    import contextlib
    import ctypes
    import sys
    import types

    if "antenv.axon_hooks" in sys.modules:
        return
    so_path = "/opt/axon/libaxon_pjrt.so"
    lib = ctypes.CDLL(so_path)
    lib.axon_start_nrt_profile.argtypes = [ctypes.POINTER(ctypes.c_int64),
                                           ctypes.c_size_t]
    lib.axon_start_nrt_profile.restype = ctypes.c_int64
    lib.axon_stop_nrt_profile.argtypes = [ctypes.c_char_p]
    lib.axon_stop_nrt_profile.restype = ctypes.c_int64

    @contextlib.contextmanager
    def _hook(output_dir, device_ids):
        import jax
        jax.devices()
        if device_ids:
            ids = (ctypes.c_int64 * len(device_ids))(*device_ids)
            rc = lib.axon_start_nrt_profile(ids, len(device_ids))
        else:
            rc = lib.axon_start_nrt_profile(None, 0)
        if rc != 0:
            raise RuntimeError(f"axon_start_nrt_profile rc={rc}")
        try:
            yield
        finally:
            n = lib.axon_stop_nrt_profile(str(output_dir).encode())
            print(f"profile: {n} file(s) written to {output_dir}",
                  file=sys.stderr)

    mod = types.ModuleType("antenv.axon_hooks")
    mod.get_axon_ntff_profile_hook = lambda: _hook
    mod.set_axon_ntff_profile_hook = lambda h: None
    sys.modules["antenv.axon_hooks"] = mod
    import antenv
    antenv.axon_hooks = mod


def kernel(primary_input, W0, b0, W1, b1, W_out, b_out, _trace=False,
           _trace_cores=None):
    from concourse import bass_utils

    if _trace:
        _install_ntff_hook()

    nc, _ = _build_program()
    in_maps = _make_in_maps(primary_input, W0, b0, W1, b1, W_out, b_out)
    res = bass_utils.run_bass_kernel_spmd(
        nc, in_maps, core_ids=list(range(N_CORES)),
        trace=_trace, trace_cores=_trace_cores)
    out = np.empty((B_FULL, N_OUT), dtype=np.float32)
    for core in range(N_CORES):
        out[core * BC:(core + 1) * BC] = res.results[core]["out"].T
    if _trace:
        kernel._last_results = res
    return out



# revision 30
# speedup vs baseline: 1.2572x; 1.1400x over previous
"""Capsule-network kernel for 8x TRN2 NeuronCores (data-parallel over batch).

Reference computation (see problem):
  prim = primary_input.reshape(B, 8, 1024)
  prev = zeros(B, 4096)
  for col in 0..3:
    # layer0: inp = [prim_t, x_t, col] (1537) @ W0 -> relu -> flat -> roll(-128)
    # layer1: inp = [x_t, col] (513) @ W1 -> relu -> flat -> roll(+128)
  out = prev @ W_out + b_out

Kernel strategy (per core, batch shard Bc=512):
  - Everything on-chip is FEATURE-MAJOR: tiles are [128 features, Bc batch].
    ROLL=128 == partition count, so rolls are free tile re-indexings.
  - The scalar `col` concat input contributes col*W[last_row] to the
    pre-activation -> folded into per-col biases (computed on host).
  - P = prim @ W0[0:1024] is col-invariant -> computed once (phase 1),
    kept in SBUF as bf16, added during the layer0 drain each col.
  - col 0 layer0 has x=0 -> out = relu(P + b0): no matmuls at all.
  - Matmuls run as bf16 (same 1 col/cycle PE rate as fp32r, but
    the 2-byte LDWEIGHTS hides fully: measured cadence ~216ns vs
    ~227ns for fp32r).  Activations/weights bf16, psum fp32.
  - HW measurement: an fp32r matmul whose stationary weights differ from
    the previous matmul costs ~252ns; same-weights runs cost ~226.7ns.
    So layers are swept (fo, k) outer / row t inner: 8 consecutive
    matmuls share one weight tile (one sweep = 8 psum banks).
  - 6 dummy matmuls at t=0 (on a memset tile) ramp the PE out of its
    low p-state during the initial DMA wait.
"""

import numpy as np

# ---- problem constants (hardcoded; kernel.py must be self-contained) ----
B_FULL = 4096
D_IN = 8192
T = 8            # NUM_TALL
NW = 4           # NUM_WIDE
F = 512          # feature size per capsule row
ROLL = 128
N_CORES = 8
BC = B_FULL // N_CORES   # per-core batch = 512
S = (F * T) // 128       # state feature tiles = 32
KP = (D_IN // T) // 128  # prim k-tiles per capsule row = 8
KX = F // 128            # x k-tiles = 4
FO = F // 128            # output feature tiles per row-layer = 4
N_OUT = 10
N_WARM = 6               # dummy p-state warmup matmuls

_CACHE = {}


def _build_program():
    """Build (and cache) the single-core Bass program. Same program runs
    SPMD on all 8 cores with different batch shards."""
    if "nc" in _CACHE:
        return _CACHE["nc"], _CACHE["names"]

    from contextlib import ExitStack

    import concourse.tile as tile
    from concourse import bacc, mybir

    f32 = mybir.dt.float32
    f8 = mybir.dt.float8e4
    f32r = mybir.dt.float32r
    bf16 = mybir.dt.bfloat16
    AF = mybir.ActivationFunctionType
    DR = mybir.MatmulPerfMode.DoubleRow
    ADD = mybir.AluOpType.add

    nc = bacc.Bacc("TRN2", target_bir_lowering=False, debug=False,
                   num_devices=N_CORES)

    # prim + w0p travel and multiply as bf16: phase-1 is the only
    # DMA-heavy span (16.8MB of prim in fp32 saturates the 360 GB/s DMA
    # system during the cold start); halving the bytes costs ~26ns/matmul
    # of bf16 LDWEIGHTS overhead on the 256 P-matmuls but removes all
    # DMA-starvation stalls.
    prim_d = nc.dram_tensor("prim_t", [D_IN, BC], bf16, kind="ExternalInput").ap()
    w0p_d = nc.dram_tensor("w0p", [KP * 128, F], bf16, kind="ExternalInput").ap()
    w0x_d = nc.dram_tensor("w0x", [F, F], bf16, kind="ExternalInput").ap()
    w1x_d = nc.dram_tensor("w1x", [F, F], bf16, kind="ExternalInput").ap()
    wout_d = nc.dram_tensor("wout_packed", [128, S * N_OUT], bf16,
                            kind="ExternalInput").ap()
    w1x8_d = nc.dram_tensor("w1x8", [(KX // 2) * 128, 2, F], f8,
                            kind="ExternalInput").ap()
    w0x8_d = nc.dram_tensor("w0x8", [(KX // 2) * 128, 2, F], f8,
                            kind="ExternalInput").ap()
    bias116_d = nc.dram_tensor("bias1_16", [128, NW * FO], f32,
                               kind="ExternalInput").ap()
    bias016_d = nc.dram_tensor("bias0_16", [128, NW * FO], f32,
                               kind="ExternalInput").ap()
    bias0_d = nc.dram_tensor("bias0", [128, NW * FO], f32, kind="ExternalInput").ap()
    bias1_d = nc.dram_tensor("bias1", [128, NW * FO], f32, kind="ExternalInput").ap()
    bout_d = nc.dram_tensor("bout", [N_OUT, 1], f32, kind="ExternalInput").ap()
    out_d = nc.dram_tensor("out", [N_OUT, BC], f32, kind="ExternalOutput").ap()

    with tile.TileContext(nc) as tc, ExitStack() as ctx:
        const = ctx.enter_context(tc.tile_pool(name="const", bufs=1))
        state = ctx.enter_context(tc.tile_pool(name="state", bufs=1))
        cpool = ctx.enter_context(tc.tile_pool(name="cpool", bufs=40))
        prim_pool = ctx.enter_context(tc.tile_pool(name="primp", bufs=12))
        ppool = ctx.enter_context(tc.tile_pool(name="psum", bufs=8, space="PSUM"))

        # ---- constants ----
        w0p_sb = [const.tile([128, F], bf16, name=f"w0p{k}", tag=f"w0p{k}")
                  for k in range(KP)]
        w0x_sb = [const.tile([128, F], bf16, name=f"w0x{k}", tag=f"w0x{k}")
                  for k in range(KX)]
        w1x_sb = [const.tile([128, F], bf16, name=f"w1x{k}", tag=f"w1x{k}")
                  for k in range(KX)]
        wout_sb = const.tile([128, S * N_OUT], bf16, name="wout_sb", tag="wout")
        w1x8_sb = [const.tile([128, 2, F], f8, name=f"w1x8_{q}", tag=f"w1x8_{q}")
                   for q in range(KX // 2)]
        w0x8_sb = [const.tile([128, 2, F], f8, name=f"w0x8_{q}", tag=f"w0x8_{q}")
                   for q in range(KX // 2)]
        bias116_sb = const.tile([128, NW * FO], f32, name="bias116_sb",
                                tag="bias116")
        bias016_sb = const.tile([128, NW * FO], f32, name="bias016_sb",
                                tag="bias016")
        bias0_sb = const.tile([128, NW * FO], f32, name="bias0_sb", tag="bias0")
        bias1_sb = const.tile([128, NW * FO], f32, name="bias1_sb", tag="bias1")
        bout_sb = const.tile([N_OUT, 1], f32, name="bout_sb", tag="bout")
        warm_sb = const.tile([128, 128], f32, name="warm_sb", tag="warm")

        # ---- persistent state ----
        A = [state.tile([128, BC], bf16, name=f"state_a{i}", tag=f"A{i}")
             for i in range(S)]
        P = [state.tile([128, BC], bf16, name=f"state_p{i}", tag=f"P{i}")
             for i in range(S)]

        # ---- p-state warmup: dummy matmuls on a memset tile ----
        # (fp32 runs at 4 cycles/row so a 128-wide moving dim gives
        # ~213-790ns per dummy across the ramp)
        nc.vector.memset(warm_sb[:], 0.0)
        for i in range(N_WARM):
            ps = ppool.tile([128, BC], f32, name=f"warm{i}", tag="mm")
            nc.tensor.matmul(ps[0:128, 0:128], warm_sb[:], warm_sb[:],
                             start=True, stop=True)

        def load_deferred_consts(gi):
            # late-needed constants ride the idle gpsimd (SWDGE) queue so
            # the sync/scalar queues keep streaming prim.  w1x must be
            # resident by ~15us (first col0-layer1 chunk), so it goes out
            # at gi=0 -- the gpsimd queue has nothing else and issues it
            # at t~1us.
            if gi == 0:
                nc.sync.dma_start(bias0_sb[:], bias0_d[:, :])
                nc.gpsimd.dma_start(bias1_sb[:], bias1_d[:, :])
                nc.gpsimd.dma_start(bout_sb[:], bout_d[:, :])
                for k in range(KX):
                    nc.gpsimd.dma_start(w1x_sb[k][:], w1x_d[k * 128:(k + 1) * 128, :])
                for q in range(KX // 2):
                    nc.gpsimd.dma_start(w1x8_sb[q][:],
                                        w1x8_d[q * 128:(q + 1) * 128])
                nc.gpsimd.dma_start(bias016_sb[:], bias016_d[:, :])
                nc.gpsimd.dma_start(bias116_sb[:], bias116_d[:, :])
                for q in range(KX // 2):
                    nc.gpsimd.dma_start(w0x8_sb[q][:],
                                        w0x8_d[q * 128:(q + 1) * 128])
            elif gi == 1:
                for k in range(KX):
                    nc.gpsimd.dma_start(w0x_sb[k][:], w0x_d[k * 128:(k + 1) * 128, :])
            elif gi == 5:
                nc.gpsimd.dma_start(wout_sb[:], wout_d[:, :])

        C = [None] * S  # col-current layer0 outputs (cpool ring tiles)

        def _pair_of(j):
            # fp8 DoubleRow pair p holds tiles (2p+1, 2p+2 % 32)
            return ((j - 1) % S) // 2, 1 - (j % 2)

        def layer1_fp8(c, C8p, out8=None):
            # raw fp8 DoubleRow layer1: K=256 per instruction (2 chunks),
            # x = fp8(16*relu), W = fp8(32*W1x) -> psum scale 512.
            # Error is diluted by downstream relu/P mixing (measured).
            for fo in range(FO):
                pss = [ppool.tile([128, BC], f32, name=f"p8_{c}_{fo}_{t}",
                                  tag="mm") for t in range(T)]
                for q in range(KX // 2):
                    w_ap = w1x8_sb[q][:, :, fo * 128:(fo + 1) * 128]
                    for t in range(T):
                        nc.tensor.matmul(
                            pss[t][:], w_ap, C8p[(2 * t + q) % (S // 2)][:, :, :],
                            start=(q == 0), stop=(q == KX // 2 - 1),
                            perf_mode=DR)
                for t in range(T):
                    j = t * FO + fo
                    if out8 is None:
                        dst, b1ap, sc = (A[j][:],
                                         bias1_sb[:, c * FO + fo:c * FO + fo + 1],
                                         1.0 / 512)
                    else:
                        p, h = _pair_of(j)
                        dst, b1ap, sc = (out8[p][:, h, :],
                                         bias116_sb[:, c * FO + fo:c * FO + fo + 1],
                                         1.0 / 32)
                    if t % 2 == 0:
                        # ACT: relu(psum*sc + b) in one op
                        nc.scalar.activation(dst, pss[t][:], AF.Relu,
                                             bias=b1ap, scale=sc)
                    else:
                        # DVE 2-op: u = psum*sc + b (bf16), then max(u,0)
                        # -- halves the ACT wall on fp8 layers
                        u = cpool.tile([128, BC], bf16, name=f"u8_{c}_{j}",
                                       tag="C")
                        nc.vector.tensor_scalar(u[:], pss[t][:], sc, b1ap,
                                                mybir.AluOpType.mult, ADD)
                        nc.vector.tensor_scalar(dst, u[:], 0.0, 0.0,
                                                mybir.AluOpType.max, ADD)



        # ==== phase 1: P[j] = prim @ W0p (col-invariant), fused col0-L0 ====
        # Per-row groups (4 psum banks each): bf16 LDWEIGHTS hides fully
        # so weight-run batching is unnecessary, and small groups smooth
        # the psum-drain handoff between groups.
        # Drains: P copy (DVE, bf16) + col0-L0 C = relu(P+b0) (ACT).
        groups = [(t,) for t in range(T)]
        C8 = [cpool.tile([128, 2, BC], f8, name=f"c8a_{p}", tag="C")
              for p in range(S // 2)]

        def prim_dma(t, k):
            # one k-tile per DMA, full-width: the DMA system is
            # descriptor-line-rate limited, so 1KB lines (full bf16 rows)
            # move twice the bytes per line vs split halves.  Row 0 and
            # odd rows ride the scalar queue (it starts issuing ~5us
            # before sync, which carries the TileContext preamble).
            g = t * KP + k
            tile_ = prim_pool.tile([128, BC], bf16, name=f"prim_{g}",
                                   tag="prim")
            # scalar (ACT engine: also runs drains) only carries row 0,
            # interleaved with w0p.  Sync takes rows 1/2/4/6 (row 1 first,
            # right after bias0, so it lands before its ~14us deadline);
            # gpsimd takes rows 3/5/7 after the small deferred consts.
            if t == 0:
                q = nc.scalar
            elif t in (1, 2, 4, 6):
                q = nc.sync
            else:
                q = nc.gpsimd
            q.dma_start(tile_[:], prim_d[g * 128:(g + 1) * 128, :])
            return tile_

        def layer1_chunk(c, rows):
            # layer1 for a subset of rows (weight run-of-len(rows)).
            # A[4t+fo] = relu(W1x.T C + b1c);  C k-tile = C[(4t+k+1)%S]
            for fo in range(FO):
                pss = {t: ppool.tile([128, BC], f32, name=f"ps1_{c}_{fo}_{t}",
                                     tag="mm") for t in rows}
                for k in range(KX):
                    w_ap = w1x_sb[k][:, fo * 128:(fo + 1) * 128]
                    for t in rows:
                        nc.tensor.matmul(
                            pss[t][:], w_ap, C[(t * FO + k + 1) % S][:],
                            start=(k == 0), stop=(k == KX - 1))
                b1ap = bias1_sb[:, c * FO + fo:c * FO + fo + 1]
                for t in rows:
                    j = t * FO + fo
                    if t % 2 == 0:
                        nc.scalar.activation(A[j][:], pss[t][:], AF.Relu,
                                             bias=b1ap)
                    else:
                        # relu(psum + bias) on DVE: (psum add bias) max 0
                        nc.vector.tensor_scalar(A[j][:], pss[t][:], b1ap, 0.0,
                                                ADD, mybir.AluOpType.max)

        for gi, grp in enumerate(groups):
            pss = {}
            for t in grp:
                for fo in range(FO):
                    pss[(t, fo)] = ppool.tile([128, BC], f32,
                                              name=f"ps_p1_{t}_{fo}", tag="mm")
            pt = {}
            if gi == 0:
                # interleave w0p with row 0's tiles on scalar so the
                # k-th matmul's pair (w0p[k], prim(0,k)) lands together
                for k in range(KP):
                    nc.scalar.dma_start(w0p_sb[k][:],
                                        w0p_d[k * 128:(k + 1) * 128, :])
                    pt[(0, k)] = prim_dma(0, k)
            else:
                for k in range(KP):
                    for t in grp:
                        pt[(t, k)] = prim_dma(t, k)
            for k in range(KP):
                for fo in range(FO):
                    for t in grp:
                        nc.tensor.matmul(
                            pss[(t, fo)][:],
                            w0p_sb[k][:, fo * 128:(fo + 1) * 128],
                            pt[(t, k)][:],
                            start=(k == 0), stop=(k == KP - 1))
            load_deferred_consts(gi)
            for t in grp:
                for fo in range(FO):
                    j = t * FO + fo
                    nc.vector.tensor_copy(P[j][:], pss[(t, fo)][:])
                    # col0-L1 runs raw fp8 DoubleRow: C stored as fp8
                    # pair-tile halves at scale 16
                    p, h = _pair_of(j)
                    nc.scalar.activation(C8[p][:, h, :], pss[(t, fo)][:],
                                         AF.Relu,
                                         bias=bias016_sb[:, fo:fo + 1],
                                         scale=16.0)
        # col-0 layer1 with full run-of-8 weight reuse (bf16 phase-1 DMA
        # leaves enough bandwidth slack that no absorber work is needed)
        layer1_fp8(0, C8)

        # ==== layer emitters: (fo, k) outer, t inner -> weight run-of-8 ====
        def layer0_col(c, out8=None):
            # C[4t+fo] = relu(W0x.T x + P + b0c);  x k-tile = A[(4t+k-1)%S]
            # The t sweep starts at t=1: the k=0 input A[4t-1] is a fo3
            # tile of the previous col's layer1, and t=0 needs A[31] --
            # the very LAST drain of that col.  Rotating gives each A
            # one extra sweep-step of drain slack.
            rows = [(1 + i) % T for i in range(T)]
            for fo in range(FO):
                pss = {t: ppool.tile([128, BC], f32, name=f"ps0_{c}_{fo}_{t}",
                                     tag="mm") for t in rows}
                for k in range(KX):
                    w_ap = w0x_sb[k][:, fo * 128:(fo + 1) * 128]
                    for t in rows:
                        nc.tensor.matmul(
                            pss[t][:], w_ap, A[(t * FO + k - 1) % S][:],
                            start=(k == 0), stop=(k == KX - 1))
                b0ap = bias0_sb[:, c * FO + fo:c * FO + fo + 1]
                for t in rows:
                    j = t * FO + fo
                    ct = cpool.tile([128, BC], bf16, name=f"c{c}_{j}", tag="C")
                    # ct = (psum + bias0_c) + P  on DVE, then relu on ACT
                    nc.vector.scalar_tensor_tensor(
                        ct[:], pss[t][:], b0ap, P[j][:], ADD, ADD)
                    if out8 is None:
                        nc.scalar.activation(ct[:], ct[:], AF.Relu)
                        C[j] = ct
                    else:
                        # next layer1 is fp8: C8 half = fp8(16*relu(ct))
                        p, h = _pair_of(j)
                        nc.scalar.activation(out8[p][:, h, :], ct[:], AF.Relu,
                                             scale=16.0)

        def layer1_col(c):
            # A[4t+fo] = relu(W1x.T C + b1c);  C k-tile = C[(4t+k+1)%S]
            for fo in range(FO):
                pss = [ppool.tile([128, BC], f32, name=f"ps1_{c}_{fo}_{t}",
                                  tag="mm") for t in range(T)]
                for k in range(KX):
                    w_ap = w1x_sb[k][:, fo * 128:(fo + 1) * 128]
                    for t in range(T):
                        nc.tensor.matmul(
                            pss[t][:], w_ap, C[(t * FO + k + 1) % S][:],
                            start=(k == 0), stop=(k == KX - 1))
                b1ap = bias1_sb[:, c * FO + fo:c * FO + fo + 1]
                for t in range(T):
                    j = t * FO + fo
                    if t % 2 == 0:
                        nc.scalar.activation(A[j][:], pss[t][:], AF.Relu,
                                             bias=b1ap)
                    else:
                        # relu(psum + bias) on DVE: (psum add bias) max 0
                        nc.vector.tensor_scalar(A[j][:], pss[t][:], b1ap, 0.0,
                                                ADD, mybir.AluOpType.max)

        def layer0_fp8(c, A8p, out8):
            # raw fp8 DoubleRow layer0: x chunk q covers pairs (2t+q-1);
            # chunk 1 (fo1/fo2 inputs) first, chunk 0 (fo3/fo0) chases
            # the previous layer1's drains.  psum = 512*x@W0x; bias rides
            # the fp8 relu stage.
            rows = [(1 + i) % T for i in range(T)]
            for fo in range(FO):
                pss = {t: ppool.tile([128, BC], f32, name=f"p08_{c}_{fo}_{t}",
                                     tag="mm") for t in rows}
                for q in (1, 0):
                    w_ap = w0x8_sb[q][:, :, fo * 128:(fo + 1) * 128]
                    for t in rows:
                        nc.tensor.matmul(
                            pss[t][:], w_ap,
                            A8p[(2 * t + q - 1) % (S // 2)][:, :, :],
                            start=(q == 1), stop=(q == 0), perf_mode=DR)
                b0ap = bias016_sb[:, c * FO + fo:c * FO + fo + 1]
                for t in rows:
                    j = t * FO + fo
                    ct = cpool.tile([128, BC], bf16, name=f"c8t{c}_{j}", tag="C")
                    # ct = psum/512 + P (no bias yet); then fp8 relu stage
                    # C8 half = relu(16*ct + 16*b0c)
                    nc.vector.scalar_tensor_tensor(
                        ct[:], pss[t][:], 1.0 / 512, P[j][:],
                        mybir.AluOpType.mult, ADD)
                    p, h = _pair_of(j)
                    nc.scalar.activation(out8[p][:, h, :], ct[:], AF.Relu,
                                         bias=b0ap, scale=16.0)

        # ==== cols 1..3 (col-0 layer1 was interleaved into phase 1) ====
        C8b = [cpool.tile([128, 2, BC], f8, name=f"c8b_{p}", tag="C")
               for p in range(S // 2)]
        layer0_col(1, out8=C8b)
        layer1_fp8(1, C8b)
        C8c = [cpool.tile([128, 2, BC], f8, name=f"c8c_{p}", tag="C")
               for p in range(S // 2)]
        layer0_col(2, out8=C8c)
        layer1_fp8(2, C8c)
        layer0_col(3)
        layer1_col(3)

        # ---- final: out = prev @ W_out + b_out;  prev[k] = A[(k-1) % S] ----
        psf_full = ppool.tile([128, BC], f32, name="psf", tag="mm")
        psf = psf_full[0:N_OUT, :]
        # emit in col-3's A-write order (sweep fo, then t) so the
        # accumulation chain chases the layer1 drains
        n = 0
        for fo in range(FO):
            for t in range(T):
                k = (t * FO + fo + 1) % S
                nc.tensor.matmul(
                    psf[:],
                    wout_sb[:, k * N_OUT:(k + 1) * N_OUT],
                    A[(k - 1) % S][:],
                    start=(n == 0), stop=(n == S - 1))
                n += 1
        out_sb = cpool.tile([N_OUT, BC], f32, name="out_sb", tag="C")
        nc.scalar.activation(out_sb[:], psf[:], AF.Identity, bias=bout_sb[:])
        nc.sync.dma_start(out_d[:, :], out_sb[:])

    nc.compile()

    names = dict(prim="prim_t", w0p="w0p", w0x="w0x", w1x="w1x",
                 wout="wout_packed", bias0="bias0", bias1="bias1",
                 bout="bout", out="out")
    _CACHE["nc"] = nc
    _CACHE["names"] = names
    return nc, names


def _make_in_maps(primary_input, W0, b0, W1, b1, W_out, b_out):
    """Host-side sharding + layout prep (all cheap numpy except the
    feature-major transpose of the batch shards)."""
    primary_input = np.ascontiguousarray(primary_input, dtype=np.float32)
    W0 = np.asarray(W0, dtype=np.float32)
    b0 = np.asarray(b0, dtype=np.float32)
    W1 = np.asarray(W1, dtype=np.float32)
    b1 = np.asarray(b1, dtype=np.float32)
    W_out = np.asarray(W_out, dtype=np.float32)
    b_out = np.asarray(b_out, dtype=np.float32)

    import ml_dtypes
    F8NP = ml_dtypes.float8_e4m3
    ps = D_IN // T  # 1024
    w0p = np.ascontiguousarray(W0[:ps].astype(ml_dtypes.bfloat16))  # [1024, 512]
    w0x = np.ascontiguousarray(W0[ps:ps + F].astype(ml_dtypes.bfloat16))
    w0_last = W0[ps + F]                             # [512]
    w1x = np.ascontiguousarray(W1[:F].astype(ml_dtypes.bfloat16))
    w1_last = W1[F]                                  # [512]

    bias0 = np.concatenate(
        [(b0 + c * w0_last).reshape(FO, 128).T for c in range(NW)], axis=1)
    bias1 = np.concatenate(
        [(b1 + c * w1_last).reshape(FO, 128).T for c in range(NW)], axis=1)
    bias0 = np.ascontiguousarray(bias0, dtype=np.float32)   # [128, 16]
    bias1 = np.ascontiguousarray(bias1, dtype=np.float32)   # [128, 16]

    # wout_packed[p, k*10+o] = W_out[128k+p, o]
    wout_packed = np.ascontiguousarray(
        W_out.reshape(S, 128, N_OUT).transpose(1, 0, 2).reshape(128, S * N_OUT)
        .astype(ml_dtypes.bfloat16))
    bout = np.ascontiguousarray(b_out.reshape(N_OUT, 1))

    # fp8 layer1 weights: pair-packed DoubleRow chunks of 32*W1x
    w1f = (32.0 * np.asarray(W1[:F], dtype=np.float32)).astype(F8NP)
    w1x8 = np.ascontiguousarray(
        w1f.reshape(KX // 2, 2, 128, F).transpose(0, 2, 1, 3)
        .reshape((KX // 2) * 128, 2, F))
    bias0_16 = np.ascontiguousarray(16.0 * bias0)
    bias1_16 = np.ascontiguousarray(16.0 * bias1)
    w0f = (32.0 * np.asarray(W0[ps:ps + F], dtype=np.float32)).astype(F8NP)
    w0x8 = np.ascontiguousarray(
        w0f.reshape(KX // 2, 2, 128, F).transpose(0, 2, 1, 3)
        .reshape((KX // 2) * 128, 2, F))
    shared = dict(w0p=w0p, w0x=w0x, w1x=w1x, wout_packed=wout_packed,
                  bias0=bias0, bias1=bias1, bout=bout,
                  w1x8=w1x8, bias0_16=bias0_16,
                  w0x8=w0x8, bias1_16=bias1_16)
    in_maps = []
    for core in range(N_CORES):
        shard = primary_input[core * BC:(core + 1) * BC]          # [512, 8192]
        prim_t = np.ascontiguousarray(shard.T.astype(ml_dtypes.bfloat16))
        m = {"prim_t": prim_t}
        m.update(shared)
        in_maps.append(m)
    return in_maps


def _install_ntff_hook():
    """Provide antenv.axon_hooks (absent in this image) backed by ctypes
    calls into libaxon_pjrt.so, so run_bass_kernel_spmd(trace=True) can
    capture NTFF profiles. Mirrors trn_agent_boot.trn_boot."""
    import contextlib
    import ctypes
    import sys
    import types

    if "antenv.axon_hooks" in sys.modules:
        return
    so_path = "/opt/axon/libaxon_pjrt.so"
    lib = ctypes.CDLL(so_path)
    lib.axon_start_nrt_profile.argtypes = [ctypes.POINTER(ctypes.c_int64),
                                           ctypes.c_size_t]
    lib.axon_start_nrt_profile.restype = ctypes.c_int64
    lib.axon_stop_nrt_profile.argtypes = [ctypes.c_char_p]
    lib.axon_stop_nrt_profile.restype = ctypes.c_int64

    @contextlib.contextmanager
    def _hook(output_dir, device_ids):
        import jax
        jax.devices()
        if device_ids:
            ids = (ctypes.c_int64 * len(device_ids))(*device_ids)
            rc = lib.axon_start_nrt_profile(ids, len(device_ids))
        else:
            rc = lib.axon_start_nrt_profile(None, 0)
        if rc != 0:
            raise RuntimeError(f"axon_start_nrt_profile rc={rc}")
        try:
            yield
        finally:
            n = lib.axon_stop_nrt_profile(str(output_dir).encode())
            print(f"profile: {n} file(s) written to {output_dir}",
                  file=sys.stderr)

    mod = types.ModuleType("antenv.axon_hooks")
    mod.get_axon_ntff_profile_hook = lambda: _hook
    mod.set_axon_ntff_profile_hook = lambda h: None
    sys.modules["antenv.axon_hooks"] = mod
    import antenv
    antenv.axon_hooks = mod


def kernel(primary_input, W0, b0, W1, b1, W_out, b_out, _trace=False,
           _trace_cores=None):
    from concourse import bass_utils

    if _trace:
        _install_ntff_hook()

    nc, _ = _build_program()
    in_maps = _make_in_maps(primary_input, W0, b0, W1, b1, W_out, b_out)
    res = bass_utils.run_bass_kernel_spmd(
        nc, in_maps, core_ids=list(range(N_CORES)),
        trace=_trace, trace_cores=_trace_cores)
    out = np.empty((B_FULL, N_OUT), dtype=np.float32)
    for core in range(N_CORES):
        out[core * BC:(core + 1) * BC] = res.results[core]["out"].T
    if _trace:
        kernel._last_results = res
    return out

